# revision 45
# baseline (speedup 1.0000x reference)
"""MoE model via global vocab-pair bucketing + per-core chunk tables on 8 TRN2
cores.

v5 reworks v4's per-core bucketing into a GLOBAL (i0//128, i1//128) pair
bucketing: the host assigns each of the 16 chunk-pair classes to a core (2 per
core, sharing the i0 chunk), so each core precomputes only the 3 vocab-chunk
tables its tokens can touch (T0[c0], T1[c1a], T1[c1b]) instead of all 8 —
cutting the T = emb @ W1 precompute from 278k to ~104k PE cycles — and every
supertile is pure (2 selection matmuls per feature chunk, no mixed spill
tiles).

The main loop is EXPERT-PHASE-MAJOR: phase fb streams W1 block fb (2 x 2 MB),
builds the fb-slice of the 3 chunk tables, then for every supertile does the
8-fc selection + paired silu + expert-fb W2 + gate-combine into a per-tile
fp32 accumulator. Selection work on block 0 therefore overlaps the DMA stream
of blocks 1-3.

Other changes vs v4:
  - one-hot masks come from the host (index marshalling), removing the
    x-broadcast K=1 matmuls and the DVE compares;
  - softmax uses reciprocal_approx_fast (5x faster than DVE reciprocal, which
    stalled the PE ~1us per supertile) and gates are normalized BEFORE the
    per-expert broadcast, dropping the 128-row reciprocal broadcast and the
    final combine multiply;
  - gate-broadcast evac moved from ACT to DVE (ACT is near-saturated by the
    paired silu evacs in the phase loop).
"""

import os
import numpy as np
import ml_dtypes

import concourse.bass as bass
import concourse.mybir as mybir
import concourse.tile as tile
from concourse.bass_utils import run_bass_kernel_spmd

BF16 = ml_dtypes.bfloat16
FP8 = ml_dtypes.float8_e4m3

B = 65536
V = 512
D = 1024
IN = 2048
E = 4
OUT = 128
NCORES = 8
F = E * D                 # 4096 features, expert-major (f = e*1024 + d)
KC = D // 128             # 8 contraction chunks per table
FB = 4                    # W1 feature blocks (1024 feats each == one expert)
NG = V // 128             # 4 vocab chunks per table

LAST_EXEC_NS = None       # set when BASSMOE_TRACE=1


def _legalize_waits(nc, max_waits=1):
    """This walrus build rejects instructions carrying more than ~1 sync-wait
    command; hoist all but the last wait onto single-wait NoOps."""
    for f in nc.m.functions:
        for bb in f.blocks:
            insts = bb.instructions
            if not any(
                inst.sync_info is not None and len(inst.sync_info.on_wait) > max_waits
                for inst in insts
            ):
                continue
            new = []
            for inst in insts:
                si = inst.sync_info
                waits = list(si.on_wait) if si is not None else []
                if len(waits) > max_waits:
                    for w in waits[:-max_waits]:
                        nop = mybir.InstNoOp(
                            name=f"legw-{nc.next_id()}", ins=[], outs=[]
                        )
                        nop.engine = inst.engine
                        nop.sync_info = mybir.SyncInfo(on_wait=[w], on_update=[])
                        new.append(nop)
                    inst.sync_info = mybir.SyncInfo(
                        on_wait=waits[-max_waits:], on_update=list(si.on_update)
                    )
                new.append(inst)
            bb.instructions = new


def build_program(visits, S, legalize=True):
    """visits: list of (offset, width, jk) with jk in {1, 2} naming which T1
    chunk table the supertile's i1 one-hots select from."""
    dt = mybir.dt
    f32, bf16 = dt.float32, dt.bfloat16
    AF = mybir.ActivationFunctionType
    ALU = mybir.AluOpType

    nc = bass.Bass()

    fp8 = dt.float8e4
    # one-hot masks are exact in fp8: halves the startup-critical DMA bytes
    m0d = nc.dram_tensor("m0", [128, S], fp8, kind="ExternalInput")
    m1d = nc.dram_tensor("m1", [128, S], fp8, kind="ExternalInput")
    # embc[p, j, kc, v] = emb_tab(j)[chunk(j)*128 + v, kc*128 + p]
    embtd = nc.dram_tensor("embc", [128, 3, KC, 128], bf16, kind="ExternalInput")
    # w1m[t, fb, kc, p, ff] = W1flat[t*1024 + kc*128 + p, fb*1024 + ff]
    w1d = nc.dram_tensor("w1m", [2, FB, KC, 128, 1024], bf16, kind="ExternalInput")
    b1rd = nc.dram_tensor("b1row", [1, F], bf16, kind="ExternalInput")
    bgrd = nc.dram_tensor("bgrow", [1, E], bf16, kind="ExternalInput")
    wgd = nc.dram_tensor("wgm", [128, 2, KC, E], bf16, kind="ExternalInput")
    w2d = nc.dram_tensor("w2s", [128, E, KC, OUT], bf16, kind="ExternalInput")
    b2d = nc.dram_tensor("b2s", [128, E], f32, kind="ExternalInput")
    seld = nc.dram_tensor("sels", [128, E, 128], bf16, kind="ExternalInput")
    outd = nc.dram_tensor("out", [128, S], f32, kind="ExternalOutput")

    with tile.TileContext(nc) as tc:
        with (
            tc.tile_pool(name="const", bufs=1) as cpool,
            tc.tile_pool(name="w1st", bufs=2) as w1pool,
            tc.tile_pool(name="tt", bufs=2) as tpool,
            tc.tile_pool(name="hs", bufs=2) as hpool,
            tc.tile_pool(name="sm", bufs=2) as smpool,
            tc.tile_pool(name="gate", bufs=1) as gatepool,
            tc.tile_pool(name="accp", bufs=1) as apool,
            tc.tile_pool(name="tmpp", bufs=2) as tmpool,
            tc.tile_pool(name="gsc", bufs=2) as gspool,
            tc.tile_pool(name="pmm", bufs=2, space="PSUM") as pmm,
            tc.tile_pool(name="peo", bufs=1, space="PSUM") as peo,
            tc.tile_pool(name="pmisc", bufs=3, space="PSUM") as pmisc,
        ):
            # --- prologue DMAs, ordered by first use ---
            wg_sb = cpool.tile([128, 2, KC, E], bf16)
            nc.sync.dma_start(wg_sb[:], wgd[:])
            bgr_sb = cpool.tile([1, E], bf16)
            nc.sync.dma_start(bgr_sb[:], bgrd[:])
            embc_sb = cpool.tile([128, 3, KC, 128], bf16)
            w1t0 = w1pool.tile([128, KC, 1024], bf16, tag="w1")
            for kc in range(KC):
                nc.sync.dma_start(embc_sb[:, 0, kc], embtd[:, 0, kc])
                nc.sync.dma_start(w1t0[:, kc, :], w1d[0, 0, kc])
            b1r_sb = cpool.tile([1, F], bf16)
            nc.sync.dma_start(b1r_sb[:], b1rd[:])

            ones128_bf = cpool.tile([1, 128], bf16)
            nc.vector.memset(ones128_bf[:], 1.0)
            # all-ones [128,128]: the sum-exp matmul then lands the sumexp
            # replicated on every output partition (broadcast for free)
            onessq = cpool.tile([128, 128], bf16)
            nc.vector.memset(onessq[:], 1.0)
            # exp'd gating chunk tables, padded to 128 stationary columns
            # (zero cols 4..127) so every main-loop matmul keeps the
            # (128,128) PE tile config — no quadrant-switch drains
            g128_sb = cpool.tile([128, 3, 128], bf16)
            nc.vector.memset(g128_sb[:], 0.0)

            # w1t1 + the remaining embc chunks stream per-kc interleaved so
            # the fb0/j1 precompute can chase the DMA
            m0_sb = cpool.tile([128, S], fp8)
            m1_sb = cpool.tile([128, S], fp8)
            w1t1 = w1pool.tile([128, KC, 1024], bf16, tag="w1")
            for kc in range(KC):
                nc.sync.dma_start(w1t1[:, kc, :], w1d[1, 0, kc])
                nc.sync.dma_start(embc_sb[:, 1, kc], embtd[:, 1, kc])
            # first mask pieces beat embc chunk 2: the phase-0 bucket-a visits
            # need them ~30us before any bucket-b (chunk 2) work
            mp = min(1024, S)
            nc.sync.dma_start(m0_sb[:, 0:mp], m0d[:, 0:mp])
            nc.sync.dma_start(m1_sb[:, 0:mp], m1d[:, 0:mp])
            nc.sync.dma_start(embc_sb[:, 2], embtd[:, 2])

            def emit_g(j):
                # one Exp table chunk; all Exp run before any Silu so the ACT
                # table set loads exactly once each
                tj = 0 if j == 0 else 1
                psg = pmisc.tile([128, E], f32, tag="misc")
                for kc in range(KC):
                    nc.tensor.matmul(
                        psg[:],
                        embc_sb[:, j, kc, :],
                        wg_sb[:, tj, kc, :],
                        start=(kc == 0),
                        stop=(kc == KC - 1 and j != 0),
                    )
                if j == 0:
                    # fold bg into chunk 0's table: psg += ones(v) x bg
                    nc.tensor.matmul(
                        psg[:], ones128_bf[:], bgr_sb[:], start=False, stop=True
                    )
                nc.scalar.activation(g128_sb[:, j, 0:E], psg[:], AF.Exp, bias=0.0)

            # W2 weights beat the mask remainder: the first visit's W2 runs at
            # ~31us while later mask pieces have ~10us of slack per piece
            w2_sb = cpool.tile([128, E, KC, OUT], bf16)
            nc.sync.dma_start(w2_sb[:], w2d[:])
            b2_sb = cpool.tile([128, E], f32)
            nc.sync.dma_start(b2_sb[:], b2d[:])
            sel_sb = cpool.tile([128, E, 128], bf16)
            nc.sync.dma_start(sel_sb[:], seld[:])

            for c0 in range(1024, S, 1024):
                c1 = min(S, c0 + 1024)
                nc.sync.dma_start(m0_sb[:, c0:c1], m0d[:, c0:c1])
                nc.sync.dma_start(m1_sb[:, c0:c1], m1d[:, c0:c1])

            acc = {}
            gates = {}
            gparts = {}

            def emit_gating_part1(vi, off, w, jk):
                # exp-gate selections land on psum rows 0..3 (zero-padded
                # stationary cols keep rows 4..127 at exactly 0)
                psa = pmisc.tile([128, w], f32, tag="misc")
                nc.tensor.matmul(
                    psa[:], g128_sb[:, 0, :], m0_sb[:, off : off + w],
                    start=True, stop=True,
                )
                psb = pmisc.tile([128, w], f32, tag="misc")
                nc.tensor.matmul(
                    psb[:], g128_sb[:, jk, :], m1_sb[:, off : off + w],
                    start=True, stop=True,
                )
                sa = smpool.tile([128, 512], f32, tag="s0")
                nc.vector.tensor_copy(sa[:, 0:w], psa[:])
                expt = smpool.tile([128, 512], bf16, tag="expt")
                nc.vector.tensor_tensor(expt[:, 0:w], sa[:, 0:w], psb[:], ALU.mult)
                gparts[vi] = expt

            def emit_gating_part2a(vi, off, w, jk):
                # sum-exp via all-ones stationary: sumexp appears replicated
                # on every psum row, so 1/sumexp needs no broadcast matmul
                expt = gparts[vi]
                sp = pmisc.tile([128, w], f32, tag="misc")
                nc.tensor.matmul(
                    sp[:], onessq[:], expt[:, 0:w], start=True, stop=True
                )
                rec = smpool.tile([128, 512], f32, tag="rec")
                nc.vector.reciprocal_approx_fast(rec[:, 0:w], sp[:])
                gparts[vi] = (expt, rec)

            def emit_gating_part2b(vi, off, w, jk):
                expt, rec = gparts.pop(vi)
                gt = gatepool.tile([128, 512], bf16, tag=f"gate{vi}")
                nc.vector.tensor_tensor(gt[:, 0:w], expt[:, 0:w], rec[:, 0:w], ALU.mult)
                gates[vi] = gt

            def emit_w2(eop, fb, w, hpair, p):
                for half in range(2):
                    dc = p * 2 + half
                    nc.tensor.matmul(
                        eop[:, 0:w],
                        w2_sb[:, fb, dc, :],
                        hpair[:, half, 0:w],
                        start=(dc == 0),
                        stop=(dc == KC - 1),
                    )

            def emit_combine(p):
                # (eop + b2) * gate -> acc, for the CARRIED visit p
                fb = p["fb"]
                vi, w, off, eop, gs = p["vi"], p["w"], p["off"], p["eop"], p["gs"]
                if fb == 0:
                    a = apool.tile([128, 512], f32, tag=f"acc{vi}")
                    acc[vi] = a
                    nc.vector.scalar_tensor_tensor(
                        a[:, 0:w], eop[:, 0:w], b2_sb[:, fb : fb + 1],
                        gs[:, 0:w], ALU.add, ALU.mult,
                    )
                else:
                    tmp = tmpool.tile([128, 512], f32, tag="tmp")
                    nc.vector.scalar_tensor_tensor(
                        tmp[:, 0:w], eop[:, 0:w], b2_sb[:, fb : fb + 1],
                        gs[:, 0:w], ALU.add, ALU.mult,
                    )
                    nc.vector.tensor_add(acc[vi][:, 0:w], acc[vi][:, 0:w], tmp[:, 0:w])
                if fb == FB - 1:
                    nc.sync.dma_start(outd[:, off : off + w], acc[vi][:, 0:w])

            def flush_pending(p):
                if p is None:
                    return
                emit_w2(p["eop"], p["fb"], p["w"], p["hs"][2], 2)
                emit_w2(p["eop"], p["fb"], p["w"], p["hs"][3], 3)
                emit_combine(p)

            def emit_visit(fb, vi, off, w, jk, t_sb, pending):
                # Software-pipelined across visits: this visit's selection
                # stream carries the PREVIOUS visit's last two W2 pairs and
                # combine, so every W2 matmul trails its silu evac by >2.5us
                # and the PE never waits on the ACT engine.
                if fb == 0:
                    emit_gating_part1(vi, off, w, jk)
                hs = []
                for pair in range(4):
                    hp = pmm.tile([128, 2, 512], f32, tag="mm")
                    # bank-interleaved order (T0h0, T0h1, T1h0, T1h1): every
                    # group-stop is followed by the other bank's stream, so
                    # the stop drain overlaps useful work
                    for half in range(2):
                        lf = pair * 2 + half
                        fs = (lf % 4) * 128
                        nc.tensor.matmul(
                            hp[:, half, 0:w],
                            t_sb[:, 0, lf // 4, fs : fs + 128],
                            m0_sb[:, off : off + w],
                            start=True, stop=False,
                        )
                    for half in range(2):
                        lf = pair * 2 + half
                        fs = (lf % 4) * 128
                        nc.tensor.matmul(
                            hp[:, half, 0:w],
                            t_sb[:, jk, lf // 4, fs : fs + 128],
                            m1_sb[:, off : off + w],
                            start=False, stop=True,
                        )
                    hpair = hpool.tile([128, 2, 512], bf16, tag=f"h{pair}")
                    nc.scalar.activation(
                        hpair[:, :, 0:w], hp[:, :, 0:w], AF.Silu, bias=0.0
                    )
                    hs.append(hpair)
                    if pair == 0 and pending is not None:
                        emit_w2(
                            pending["eop"], pending["fb"], pending["w"],
                            pending["hs"][2], 2,
                        )
                    if pair == 1 and pending is not None:
                        emit_w2(
                            pending["eop"], pending["fb"], pending["w"],
                            pending["hs"][3], 3,
                        )
                        emit_combine(pending)
                    if fb == 0 and pair == 1:
                        emit_gating_part2a(vi, off, w, jk)
                # in-visit W2 pairs 0/1 after the selection: maximum slack for
                # both the silu evacs and the peo buffer release (stt of the
                # carried visit fires ~2us before this point)
                eop = peo.tile([128, 512], f32, tag="eo")
                emit_w2(eop, fb, w, hs[0], 0)
                emit_w2(eop, fb, w, hs[1], 1)
                if fb == 0:
                    emit_gating_part2b(vi, off, w, jk)
                gp = pmisc.tile([128, w], f32, tag="misc")
                nc.tensor.matmul(
                    gp[:], sel_sb[:, fb, :], gates[vi][:, 0:w],
                    start=True, stop=True,
                )
                gs = gspool.tile([128, 512], bf16, tag="gs")
                nc.vector.tensor_copy(gs[:, 0:w], gp[:])
                return dict(vi=vi, off=off, w=w, eop=eop, hs=hs, gs=gs, fb=fb)

            # --- expert-phase-major main loop ---
            # `pending` (deferred W2 tail + combine) carries ACROSS phase
            # boundaries too: the next phase's precompute gives it slack
            pending = None
            for fb in range(FB):
                if fb == 0:
                    t0t, t1t = w1t0, w1t1
                else:
                    t0t = w1pool.tile([128, KC, 1024], bf16, tag="w1")
                    for kc in range(KC):
                        nc.sync.dma_start(t0t[:, kc, :], w1d[0, fb, kc])
                    t1t = w1pool.tile([128, KC, 1024], bf16, tag="w1")
                    for kc in range(KC):
                        nc.sync.dma_start(t1t[:, kc, :], w1d[1, fb, kc])
                # build the fb-slice of the 3 chunk tables
                t_sb = tpool.tile([128, 3, 2, 512], bf16, tag="t")

                def emit_tchunk(fb, j, w1t, t_sb):
                    ps = pmm.tile([128, 2, 512], f32, tag="mm")
                    for fp in range(2):
                        for kc in range(KC):
                            nc.tensor.matmul(
                                ps[:, fp, :],
                                embc_sb[:, j, kc, :],
                                w1t[:, kc, fp * 512 : (fp + 1) * 512],
                                start=(kc == 0),
                                stop=(kc == KC - 1 and j != 0),
                            )
                        if j == 0:
                            f0 = fb * 1024 + fp * 512
                            nc.tensor.matmul(
                                ps[:, fp, :], ones128_bf[:],
                                b1r_sb[:, f0 : f0 + 512],
                                start=False, stop=True,
                            )
                    nc.scalar.copy(t_sb[:, j, :, :], ps[:])

                # bucket-b work (chunk-2 table, G2) is deferred past the
                # bucket-a visits: it is not needed until the first jk==2
                # visit, and in phase 0 this shortens the DMA-bound startup
                if fb == 0:
                    # chunk 0 first (its W1 block is the first DMA), then the
                    # gating tables fill the PE while w1t1/embc j1,j2 stream
                    emit_tchunk(fb, 0, t0t, t_sb)
                    emit_g(0)
                    emit_tchunk(fb, 1, t1t, t_sb)
                    emit_tchunk(fb, 2, t1t, t_sb)
                    emit_g(1)
                    emit_g(2)
                else:
                    for j in range(3):
                        emit_tchunk(fb, j, t0t if j == 0 else t1t, t_sb)
                for vi, (off, w, jk) in enumerate(visits):
                    pending = emit_visit(fb, vi, off, w, jk, t_sb, pending)
            flush_pending(pending)

    if legalize:
        _legalize_waits(nc)
    mybir.codegen_inst_isa_subclasses(nc)
    return nc


def _roundup(n, m):
    return -(-n // m) * m


def assign_slots(x):
    """Global (i0//128, i1//128) bucketing: 16 classes -> 8 cores (2 each,
    sharing the i0 chunk), with identical (i0, i1) token pairs DEDUPED —
    the device computes each distinct pair once and the host scatters the
    result to every duplicate token (~11% fewer slots on uniform data)."""
    x = np.asarray(x)
    c0 = x[:, 0] // 128
    c1 = x[:, 1] // 128
    key = x[:, 0] * V + x[:, 1]
    cores = []
    for g in range(NG):
        cls = []
        for b in range(NG):
            idx = np.nonzero((c0 == g) & (c1 == b))[0]
            ukey, inv = np.unique(key[idx], return_inverse=True)
            cls.append((idx, ukey, inv))
        order = sorted(range(NG), key=lambda b: -len(cls[b][1]))
        for pa, pb in ((order[0], order[3]), (order[1], order[2])):
            if len(cls[pb][1]) > len(cls[pa][1]):
                pa, pb = pb, pa
            cores.append(dict(c0=g, c1a=pa, c1b=pb, a=cls[pa], b=cls[pb]))
    SA = _roundup(max(len(c["a"][1]) for c in cores), 64)
    SB = _roundup(max(len(c["b"][1]) for c in cores), 64)
    visits = []
    off = 0
    for span, jk in ((SA, 1), (SB, 2)):
        left = span
        while left > 0:
            w = min(512, left)
            visits.append((off, w, jk))
            off += w
            left -= w
    S = SA + SB
    slot_pairs = []   # per core: slot -> (i0, i1), -1 for pad
    tok_maps = []     # per core: (global token ids, their slots)
    for c in cores:
        si0 = np.full(S, -1, dtype=np.int64)
        si1 = np.full(S, -1, dtype=np.int64)
        (ia, ka, va), (ib, kb, vb) = c["a"], c["b"]
        si0[0 : len(ka)] = ka // V
        si1[0 : len(ka)] = ka % V
        si0[SA : SA + len(kb)] = kb // V
        si1[SA : SA + len(kb)] = kb % V
        slot_pairs.append((si0, si1))
        tok_maps.append(
            (np.concatenate([ia, ib]), np.concatenate([va, SA + vb]))
        )
    return cores, slot_pairs, tok_maps, visits, S


def marshal_inputs(x, emb0, emb1, W1, b1, W2, b2, Wg, bg, cores, slot_pairs, S):
    x = np.asarray(x)
    emb0 = np.asarray(emb0)
    emb1 = np.asarray(emb1)

    shared = {}
    # W1flat[k, f] with f = e*1024 + d (expert-major features)
    w1flat = np.asarray(W1).transpose(1, 0, 2).reshape(IN, F)
    shared["w1m"] = np.ascontiguousarray(
        w1flat.reshape(2, KC, 128, FB, 1024).transpose(0, 3, 1, 2, 4).astype(BF16)
    )
    shared["b1row"] = np.ascontiguousarray(
        np.asarray(b1).reshape(1, F).astype(BF16)
    )
    shared["bgrow"] = np.ascontiguousarray(
        np.asarray(bg).reshape(1, E).astype(BF16)
    )
    shared["wgm"] = np.ascontiguousarray(
        np.asarray(Wg).reshape(2, KC, 128, E).transpose(2, 0, 1, 3).astype(BF16)
    )
    shared["w2s"] = np.ascontiguousarray(
        np.asarray(W2).reshape(E, KC, 128, OUT).transpose(2, 0, 1, 3).astype(BF16)
    )
    shared["b2s"] = np.ascontiguousarray(np.asarray(b2).T.astype(np.float32))
    # sel128[p, e, o] = 1 iff p == e: a (128,128)-tile gate-row broadcast
    sel128 = np.zeros((128, E, 128), dtype=BF16)
    for e in range(E):
        sel128[e, e, :] = 1.0
    shared["sels"] = np.ascontiguousarray(sel128)

    in_maps = []
    for c, (si0, si1) in zip(cores, slot_pairs):
        # embc[p, j, kc, v] = emb_tab(j)[chunk(j)*128 + v, kc*128 + p]
        embc = np.empty((128, 3, KC, 128), dtype=BF16)
        for j, (tab, ch) in enumerate(
            ((emb0, c["c0"]), (emb1, c["c1a"]), (emb1, c["c1b"]))
        ):
            chunk = tab[ch * 128 : (ch + 1) * 128]  # [128v, 1024k]
            embc[:, j] = (
                chunk.reshape(128, KC, 128).transpose(2, 1, 0).astype(BF16)
            )
        # one-hot masks per slot (pad slots stay all-zero); fp8 is exact
        m0 = np.zeros((128, S), dtype=FP8)
        m1 = np.zeros((128, S), dtype=FP8)
        pos = np.nonzero(si0 >= 0)[0]
        m0[si0[pos] % 128, pos] = 1.0
        m1[si1[pos] % 128, pos] = 1.0
        in_maps.append(
            {
                "m0": m0,
                "m1": m1,
                "embc": np.ascontiguousarray(embc),
                **shared,
            }
        )
    return in_maps


def kernel(x, emb0, emb1, W1, b1, W2, b2, Wg, bg):
    global LAST_EXEC_NS
    cores, slot_pairs, tok_maps, visits, S = assign_slots(x)
    nc = build_program(visits, S)
    in_maps = marshal_inputs(
        x, emb0, emb1, W1, b1, W2, b2, Wg, bg, cores, slot_pairs, S
    )
    trace = os.environ.get("BASSMOE_TRACE", "0") == "1"
    res = run_bass_kernel_spmd(nc, in_maps, list(range(NCORES)), trace=trace)
    LAST_EXEC_NS = res.exec_time_ns
    out = np.empty((B, OUT), dtype=np.float32)
    for c in range(NCORES):
        toks, slots = tok_maps[c]
        r = res.results[c]["out"]  # [128, S]
        out[toks, :] = r[:, slots].T
    return out


# revision 54
# speedup vs baseline: 1.0211x; 1.0211x over previous
"""MoE model via global vocab-pair bucketing + per-core chunk tables on 8 TRN2
cores.

v5 reworks v4's per-core bucketing into a GLOBAL (i0//128, i1//128) pair
bucketing: the host assigns each of the 16 chunk-pair classes to a core (2 per
core, sharing the i0 chunk), so each core precomputes only the 3 vocab-chunk
tables its tokens can touch (T0[c0], T1[c1a], T1[c1b]) instead of all 8 —
cutting the T = emb @ W1 precompute from 278k to ~104k PE cycles — and every
supertile is pure (2 selection matmuls per feature chunk, no mixed spill
tiles).

The main loop is EXPERT-PHASE-MAJOR: phase fb streams W1 block fb (2 x 2 MB),
builds the fb-slice of the 3 chunk tables, then for every supertile does the
8-fc selection + paired silu + expert-fb W2 + gate-combine into a per-tile
fp32 accumulator. Selection work on block 0 therefore overlaps the DMA stream
of blocks 1-3.

Other changes vs v4:
  - one-hot masks come from the host (index marshalling), removing the
    x-broadcast K=1 matmuls and the DVE compares;
  - softmax uses reciprocal_approx_fast (5x faster than DVE reciprocal, which
    stalled the PE ~1us per supertile) and gates are normalized BEFORE the
    per-expert broadcast, dropping the 128-row reciprocal broadcast and the
    final combine multiply;
  - gate-broadcast evac moved from ACT to DVE (ACT is near-saturated by the
    paired silu evacs in the phase loop).
"""

import os
import numpy as np
import ml_dtypes

import concourse.bass as bass
import concourse.mybir as mybir
import concourse.tile as tile
from concourse.bass_utils import run_bass_kernel_spmd

BF16 = ml_dtypes.bfloat16
FP8 = ml_dtypes.float8_e4m3

B = 65536
V = 512
D = 1024
IN = 2048
E = 4
OUT = 128
NCORES = 8
F = E * D                 # 4096 features, expert-major (f = e*1024 + d)
KC = D // 128             # 8 contraction chunks per table
FB = 4                    # W1 feature blocks (1024 feats each == one expert)
NG = V // 128             # 4 vocab chunks per table

LAST_EXEC_NS = None       # set when BASSMOE_TRACE=1


def _legalize_waits(nc, max_waits=1):
    """This walrus build rejects instructions carrying more than ~1 sync-wait
    command; hoist all but the last wait onto single-wait NoOps."""
    for f in nc.m.functions:
        for bb in f.blocks:
            insts = bb.instructions
            if not any(
                inst.sync_info is not None and len(inst.sync_info.on_wait) > max_waits
                for inst in insts
            ):
                continue
            new = []
            for inst in insts:
                si = inst.sync_info
                waits = list(si.on_wait) if si is not None else []
                if len(waits) > max_waits:
                    for w in waits[:-max_waits]:
                        nop = mybir.InstNoOp(
                            name=f"legw-{nc.next_id()}", ins=[], outs=[]
                        )
                        nop.engine = inst.engine
                        nop.sync_info = mybir.SyncInfo(on_wait=[w], on_update=[])
                        new.append(nop)
                    inst.sync_info = mybir.SyncInfo(
                        on_wait=waits[-max_waits:], on_update=list(si.on_update)
                    )
                new.append(inst)
            bb.instructions = new


def build_program(visits, S, legalize=True):
    """visits: list of (offset, width, jk) with jk in {1, 2} naming which T1
    chunk table the supertile's i1 one-hots select from."""
    dt = mybir.dt
    f32, bf16 = dt.float32, dt.bfloat16
    AF = mybir.ActivationFunctionType
    ALU = mybir.AluOpType

    nc = bass.Bass()

    fp8 = dt.float8e4
    # one-hot masks are exact in fp8: halves the startup-critical DMA bytes
    m0d = nc.dram_tensor("m0", [128, S], fp8, kind="ExternalInput")
    m1d = nc.dram_tensor("m1", [128, S], fp8, kind="ExternalInput")
    # embc[p, j, kc, v] = emb_tab(j)[chunk(j)*128 + v, kc*128 + p]
    embtd = nc.dram_tensor("embc", [128, 3, KC, 128], bf16, kind="ExternalInput")
    # w1m[t, fb, p, kc, ff] = W1flat[t*1024 + kc*128 + p, fb*1024 + ff]
    # (partition-major so one coarse dma_start covers a whole block)
    w1d = nc.dram_tensor("w1m", [2, FB, 128, KC, 1024], bf16, kind="ExternalInput")
    b1rd = nc.dram_tensor("b1row", [1, F], bf16, kind="ExternalInput")
    bgrd = nc.dram_tensor("bgrow", [1, E], bf16, kind="ExternalInput")
    wgd = nc.dram_tensor("wgm", [128, 2, KC, E], bf16, kind="ExternalInput")
    w2d = nc.dram_tensor("w2s", [128, E, KC, OUT], bf16, kind="ExternalInput")
    b2d = nc.dram_tensor("b2s", [128, E], f32, kind="ExternalInput")
    seld = nc.dram_tensor("sels", [128, E, 128], bf16, kind="ExternalInput")
    outd = nc.dram_tensor("out", [128, S], f32, kind="ExternalOutput")

    with tile.TileContext(nc) as tc:
        with (
            tc.tile_pool(name="const", bufs=1) as cpool,
            tc.tile_pool(name="w1st", bufs=2) as w1pool,
            tc.tile_pool(name="tt", bufs=2) as tpool,
            tc.tile_pool(name="hs", bufs=2) as hpool,
            tc.tile_pool(name="sm", bufs=2) as smpool,
            tc.tile_pool(name="gate", bufs=1) as gatepool,
            tc.tile_pool(name="accp", bufs=1) as apool,
            tc.tile_pool(name="tmpp", bufs=2) as tmpool,
            tc.tile_pool(name="gsc", bufs=2) as gspool,
            tc.tile_pool(name="pmm", bufs=2, space="PSUM") as pmm,
            tc.tile_pool(name="peo", bufs=1, space="PSUM") as peo,
            tc.tile_pool(name="pmisc", bufs=3, space="PSUM") as pmisc,
        ):
            # --- prologue DMAs, ordered by first use ---
            wg_sb = cpool.tile([128, 2, KC, E], bf16)
            nc.sync.dma_start(wg_sb[:], wgd[:])
            bgr_sb = cpool.tile([1, E], bf16)
            nc.sync.dma_start(bgr_sb[:], bgrd[:])
            # COARSE dma_starts: each one costs ~600ns of Sync-engine issue
            # time, so the startup path uses few large transfers
            embc_sb = cpool.tile([128, 3, KC, 128], bf16)
            nc.sync.dma_start(embc_sb[:, 0], embtd[:, 0])
            w1t0 = w1pool.tile([128, KC, 1024], bf16, tag="w1")
            nc.sync.dma_start(w1t0[:, 0:4, :], w1d[0, 0, :, 0:4])
            nc.sync.dma_start(w1t0[:, 4:8, :], w1d[0, 0, :, 4:8])
            b1r_sb = cpool.tile([1, F], bf16)
            nc.sync.dma_start(b1r_sb[:], b1rd[:])

            ones128_bf = cpool.tile([1, 128], bf16)
            nc.vector.memset(ones128_bf[:], 1.0)
            # all-ones [128,128]: the sum-exp matmul then lands the sumexp
            # replicated on every output partition (broadcast for free)
            onessq = cpool.tile([128, 128], bf16)
            nc.vector.memset(onessq[:], 1.0)
            # exp'd gating chunk tables, padded to 128 stationary columns
            # (zero cols 4..127) so every main-loop matmul keeps the
            # (128,128) PE tile config — no quadrant-switch drains
            g128_sb = cpool.tile([128, 3, 128], bf16)
            nc.vector.memset(g128_sb[:], 0.0)

            # w1t1 + the remaining embc chunks + first mask pieces, ordered
            # by first use
            m0_sb = cpool.tile([128, S], fp8)
            m1_sb = cpool.tile([128, S], fp8)
            w1t1 = w1pool.tile([128, KC, 1024], bf16, tag="w1")
            nc.sync.dma_start(embc_sb[:, 1], embtd[:, 1])
            nc.sync.dma_start(w1t1[:, 0:4, :], w1d[1, 0, :, 0:4])
            nc.sync.dma_start(w1t1[:, 4:8, :], w1d[1, 0, :, 4:8])
            nc.sync.dma_start(embc_sb[:, 2], embtd[:, 2])
            mp = min(1024, S)
            nc.sync.dma_start(m0_sb[:, 0:mp], m0d[:, 0:mp])
            nc.sync.dma_start(m1_sb[:, 0:mp], m1d[:, 0:mp])

            def emit_g(j):
                # one Exp table chunk; all Exp run before any Silu so the ACT
                # table set loads exactly once each
                tj = 0 if j == 0 else 1
                psg = pmisc.tile([128, E], f32, tag="misc")
                for kc in range(KC):
                    nc.tensor.matmul(
                        psg[:],
                        embc_sb[:, j, kc, :],
                        wg_sb[:, tj, kc, :],
                        start=(kc == 0),
                        stop=(kc == KC - 1 and j != 0),
                    )
                if j == 0:
                    # fold bg into chunk 0's table: psg += ones(v) x bg
                    nc.tensor.matmul(
                        psg[:], ones128_bf[:], bgr_sb[:], start=False, stop=True
                    )
                nc.scalar.activation(g128_sb[:, j, 0:E], psg[:], AF.Exp, bias=0.0)

            # W2 weights beat the mask remainder: the first visit's W2 runs at
            # ~31us while later mask pieces have ~10us of slack per piece
            w2_sb = cpool.tile([128, E, KC, OUT], bf16)
            nc.sync.dma_start(w2_sb[:], w2d[:])
            b2_sb = cpool.tile([128, E], f32)
            nc.sync.dma_start(b2_sb[:], b2d[:])
            sel_sb = cpool.tile([128, E, 128], bf16)
            nc.sync.dma_start(sel_sb[:], seld[:])

            for c0 in range(1024, S, 2048):
                c1 = min(S, c0 + 2048)
                nc.sync.dma_start(m0_sb[:, c0:c1], m0d[:, c0:c1])
                nc.sync.dma_start(m1_sb[:, c0:c1], m1d[:, c0:c1])

            acc = {}
            gates = {}
            gparts = {}

            def emit_gating_part1(vi, off, w, jk):
                # exp-gate selections land on psum rows 0..3 (zero-padded
                # stationary cols keep rows 4..127 at exactly 0)
                psa = pmisc.tile([128, w], f32, tag="misc")
                nc.tensor.matmul(
                    psa[:], g128_sb[:, 0, :], m0_sb[:, off : off + w],
                    start=True, stop=True,
                )
                psb = pmisc.tile([128, w], f32, tag="misc")
                nc.tensor.matmul(
                    psb[:], g128_sb[:, jk, :], m1_sb[:, off : off + w],
                    start=True, stop=True,
                )
                sa = smpool.tile([128, 512], f32, tag="s0")
                nc.vector.tensor_copy(sa[:, 0:w], psa[:])
                expt = smpool.tile([128, 512], bf16, tag="expt")
                nc.vector.tensor_tensor(expt[:, 0:w], sa[:, 0:w], psb[:], ALU.mult)
                gparts[vi] = expt

            def emit_gating_part2a(vi, off, w, jk):
                # sum-exp via all-ones stationary: sumexp appears replicated
                # on every psum row, so 1/sumexp needs no broadcast matmul
                expt = gparts[vi]
                sp = pmisc.tile([128, w], f32, tag="misc")
                nc.tensor.matmul(
                    sp[:], onessq[:], expt[:, 0:w], start=True, stop=True
                )
                rec = smpool.tile([128, 512], f32, tag="rec")
                nc.vector.reciprocal_approx_fast(rec[:, 0:w], sp[:])
                gparts[vi] = (expt, rec)

            def emit_gating_part2b(vi, off, w, jk):
                expt, rec = gparts.pop(vi)
                gt = gatepool.tile([128, 512], bf16, tag=f"gate{vi}")
                nc.vector.tensor_tensor(gt[:, 0:w], expt[:, 0:w], rec[:, 0:w], ALU.mult)
                gates[vi] = gt

            def emit_w2(eop, fb, w, hpair, p):
                for half in range(2):
                    dc = p * 2 + half
                    nc.tensor.matmul(
                        eop[:, 0:w],
                        w2_sb[:, fb, dc, :],
                        hpair[:, half, 0:w],
                        start=(dc == 0),
                        stop=(dc == KC - 1),
                    )

            def emit_combine(p):
                # (eop + b2) * gate -> acc, for the CARRIED visit p
                fb = p["fb"]
                vi, w, off, eop, gs = p["vi"], p["w"], p["off"], p["eop"], p["gs"]
                if fb == 0:
                    a = apool.tile([128, 512], f32, tag=f"acc{vi}")
                    acc[vi] = a
                    nc.vector.scalar_tensor_tensor(
                        a[:, 0:w], eop[:, 0:w], b2_sb[:, fb : fb + 1],
                        gs[:, 0:w], ALU.add, ALU.mult,
                    )
                else:
                    tmp = tmpool.tile([128, 512], f32, tag="tmp")
                    nc.vector.scalar_tensor_tensor(
                        tmp[:, 0:w], eop[:, 0:w], b2_sb[:, fb : fb + 1],
                        gs[:, 0:w], ALU.add, ALU.mult,
                    )
                    nc.vector.tensor_add(acc[vi][:, 0:w], acc[vi][:, 0:w], tmp[:, 0:w])
                if fb == FB - 1:
                    nc.sync.dma_start(outd[:, off : off + w], acc[vi][:, 0:w])

            def flush_pending(p):
                if p is None:
                    return
                emit_w2(p["eop"], p["fb"], p["w"], p["hs"][2], 2)
                emit_w2(p["eop"], p["fb"], p["w"], p["hs"][3], 3)
                emit_combine(p)

            def emit_visit(fb, vi, off, w, jk, t_sb, pending):
                # Software-pipelined across visits: this visit's selection
                # stream carries the PREVIOUS visit's last two W2 pairs and
                # combine, so every W2 matmul trails its silu evac by >2.5us
                # and the PE never waits on the ACT engine.
                if fb == 0:
                    emit_gating_part1(vi, off, w, jk)
                hs = []
                for pair in range(4):
                    hp = pmm.tile([128, 2, 512], f32, tag="mm")
                    # bank-interleaved order (T0h0, T0h1, T1h0, T1h1): every
                    # group-stop is followed by the other bank's stream, so
                    # the stop drain overlaps useful work
                    for half in range(2):
                        lf = pair * 2 + half
                        fs = (lf % 4) * 128
                        nc.tensor.matmul(
                            hp[:, half, 0:w],
                            t_sb[:, 0, lf // 4, fs : fs + 128],
                            m0_sb[:, off : off + w],
                            start=True, stop=False,
                        )
                    for half in range(2):
                        lf = pair * 2 + half
                        fs = (lf % 4) * 128
                        nc.tensor.matmul(
                            hp[:, half, 0:w],
                            t_sb[:, jk, lf // 4, fs : fs + 128],
                            m1_sb[:, off : off + w],
                            start=False, stop=True,
                        )
                    hpair = hpool.tile([128, 2, 512], bf16, tag=f"h{pair}")
                    nc.scalar.activation(
                        hpair[:, :, 0:w], hp[:, :, 0:w], AF.Silu, bias=0.0
                    )
                    hs.append(hpair)
                    if pair == 0 and pending is not None:
                        emit_w2(
                            pending["eop"], pending["fb"], pending["w"],
                            pending["hs"][2], 2,
                        )
                    if pair == 1 and pending is not None:
                        emit_w2(
                            pending["eop"], pending["fb"], pending["w"],
                            pending["hs"][3], 3,
                        )
                        emit_combine(pending)
                    if fb == 0 and pair == 1:
                        emit_gating_part2a(vi, off, w, jk)
                # in-visit W2 pairs 0/1 after the selection: maximum slack for
                # both the silu evacs and the peo buffer release (stt of the
                # carried visit fires ~2us before this point)
                eop = peo.tile([128, 512], f32, tag="eo")
                emit_w2(eop, fb, w, hs[0], 0)
                emit_w2(eop, fb, w, hs[1], 1)
                if fb == 0:
                    emit_gating_part2b(vi, off, w, jk)
                gp = pmisc.tile([128, w], f32, tag="misc")
                nc.tensor.matmul(
                    gp[:], sel_sb[:, fb, :], gates[vi][:, 0:w],
                    start=True, stop=True,
                )
                gs = gspool.tile([128, 512], bf16, tag="gs")
                nc.vector.tensor_copy(gs[:, 0:w], gp[:])
                return dict(vi=vi, off=off, w=w, eop=eop, hs=hs, gs=gs, fb=fb)

            # --- expert-phase-major main loop ---
            # `pending` (deferred W2 tail + combine) carries ACROSS phase
            # boundaries too: the next phase's precompute gives it slack
            pending = None
            for fb in range(FB):
                if fb == 0:
                    t0t, t1t = w1t0, w1t1
                else:
                    t0t = w1pool.tile([128, KC, 1024], bf16, tag="w1")
                    nc.sync.dma_start(t0t[:, 0:4, :], w1d[0, fb, :, 0:4])
                    nc.sync.dma_start(t0t[:, 4:8, :], w1d[0, fb, :, 4:8])
                    t1t = w1pool.tile([128, KC, 1024], bf16, tag="w1")
                    nc.sync.dma_start(t1t[:, 0:4, :], w1d[1, fb, :, 0:4])
                    nc.sync.dma_start(t1t[:, 4:8, :], w1d[1, fb, :, 4:8])
                # build the fb-slice of the 3 chunk tables
                t_sb = tpool.tile([128, 3, 2, 512], bf16, tag="t")

                def emit_tchunk(fb, j, w1t, t_sb):
                    ps = pmm.tile([128, 2, 512], f32, tag="mm")
                    for fp in range(2):
                        for kc in range(KC):
                            nc.tensor.matmul(
                                ps[:, fp, :],
                                embc_sb[:, j, kc, :],
                                w1t[:, kc, fp * 512 : (fp + 1) * 512],
                                start=(kc == 0),
                                stop=(kc == KC - 1 and j != 0),
                            )
                        if j == 0:
                            f0 = fb * 1024 + fp * 512
                            nc.tensor.matmul(
                                ps[:, fp, :], ones128_bf[:],
                                b1r_sb[:, f0 : f0 + 512],
                                start=False, stop=True,
                            )
                    nc.scalar.copy(t_sb[:, j, :, :], ps[:])

                # bucket-b work (chunk-2 table, G2) is deferred past the
                # bucket-a visits: it is not needed until the first jk==2
                # visit, and in phase 0 this shortens the DMA-bound startup
                if fb == 0:
                    # chunk 0 first (its W1 block is the first DMA), then the
                    # gating tables fill the PE while w1t1/embc j1,j2 stream
                    emit_tchunk(fb, 0, t0t, t_sb)
                    emit_g(0)
                    emit_tchunk(fb, 1, t1t, t_sb)
                    emit_tchunk(fb, 2, t1t, t_sb)
                    emit_g(1)
                    emit_g(2)
                else:
                    for j in range(3):
                        emit_tchunk(fb, j, t0t if j == 0 else t1t, t_sb)
                for vi, (off, w, jk) in enumerate(visits):
                    pending = emit_visit(fb, vi, off, w, jk, t_sb, pending)
            flush_pending(pending)

    if legalize:
        _legalize_waits(nc)
    mybir.codegen_inst_isa_subclasses(nc)
    return nc


def _roundup(n, m):
    return -(-n // m) * m


def assign_slots(x):
    """Global (i0//128, i1//128) bucketing: 16 classes -> 8 cores (2 each,
    sharing the i0 chunk), with identical (i0, i1) token pairs DEDUPED —
    the device computes each distinct pair once and the host scatters the
    result to every duplicate token (~11% fewer slots on uniform data)."""
    x = np.asarray(x)
    c0 = x[:, 0] // 128
    c1 = x[:, 1] // 128
    key = x[:, 0] * V + x[:, 1]
    cores = []
    for g in range(NG):
        cls = []
        for b in range(NG):
            idx = np.nonzero((c0 == g) & (c1 == b))[0]
            ukey, inv = np.unique(key[idx], return_inverse=True)
            cls.append((idx, ukey, inv))
        order = sorted(range(NG), key=lambda b: -len(cls[b][1]))
        for pa, pb in ((order[0], order[3]), (order[1], order[2])):
            if len(cls[pb][1]) > len(cls[pa][1]):
                pa, pb = pb, pa
            cores.append(dict(c0=g, c1a=pa, c1b=pb, a=cls[pa], b=cls[pb]))
    SA = _roundup(max(len(c["a"][1]) for c in cores), 64)
    SB = _roundup(max(len(c["b"][1]) for c in cores), 64)
    visits = []
    off = 0
    for span, jk in ((SA, 1), (SB, 2)):
        left = span
        while left > 0:
            w = min(512, left)
            visits.append((off, w, jk))
            off += w
            left -= w
    S = SA + SB
    slot_pairs = []   # per core: slot -> (i0, i1), -1 for pad
    tok_maps = []     # per core: (global token ids, their slots)
    for c in cores:
        si0 = np.full(S, -1, dtype=np.int64)
        si1 = np.full(S, -1, dtype=np.int64)
        (ia, ka, va), (ib, kb, vb) = c["a"], c["b"]
        si0[0 : len(ka)] = ka // V
        si1[0 : len(ka)] = ka % V
        si0[SA : SA + len(kb)] = kb // V
        si1[SA : SA + len(kb)] = kb % V
        slot_pairs.append((si0, si1))
        tok_maps.append(
            (np.concatenate([ia, ib]), np.concatenate([va, SA + vb]))
        )
    return cores, slot_pairs, tok_maps, visits, S


def marshal_inputs(x, emb0, emb1, W1, b1, W2, b2, Wg, bg, cores, slot_pairs, S):
    x = np.asarray(x)
    emb0 = np.asarray(emb0)
    emb1 = np.asarray(emb1)

    shared = {}
    # W1flat[k, f] with f = e*1024 + d (expert-major features)
    w1flat = np.asarray(W1).transpose(1, 0, 2).reshape(IN, F)
    shared["w1m"] = np.ascontiguousarray(
        w1flat.reshape(2, KC, 128, FB, 1024).transpose(0, 3, 2, 1, 4).astype(BF16)
    )
    shared["b1row"] = np.ascontiguousarray(
        np.asarray(b1).reshape(1, F).astype(BF16)
    )
    shared["bgrow"] = np.ascontiguousarray(
        np.asarray(bg).reshape(1, E).astype(BF16)
    )
    shared["wgm"] = np.ascontiguousarray(
        np.asarray(Wg).reshape(2, KC, 128, E).transpose(2, 0, 1, 3).astype(BF16)
    )
    shared["w2s"] = np.ascontiguousarray(
        np.asarray(W2).reshape(E, KC, 128, OUT).transpose(2, 0, 1, 3).astype(BF16)
    )
    shared["b2s"] = np.ascontiguousarray(np.asarray(b2).T.astype(np.float32))
    # sel128[p, e, o] = 1 iff p == e: a (128,128)-tile gate-row broadcast
    sel128 = np.zeros((128, E, 128), dtype=BF16)
    for e in range(E):
        sel128[e, e, :] = 1.0
    shared["sels"] = np.ascontiguousarray(sel128)

    in_maps = []
    for c, (si0, si1) in zip(cores, slot_pairs):
        # embc[p, j, kc, v] = emb_tab(j)[chunk(j)*128 + v, kc*128 + p]
        embc = np.empty((128, 3, KC, 128), dtype=BF16)
        for j, (tab, ch) in enumerate(
            ((emb0, c["c0"]), (emb1, c["c1a"]), (emb1, c["c1b"]))
        ):
            chunk = tab[ch * 128 : (ch + 1) * 128]  # [128v, 1024k]
            embc[:, j] = (
                chunk.reshape(128, KC, 128).transpose(2, 1, 0).astype(BF16)
            )
        # one-hot masks per slot (pad slots stay all-zero); fp8 is exact
        m0 = np.zeros((128, S), dtype=FP8)
        m1 = np.zeros((128, S), dtype=FP8)
        pos = np.nonzero(si0 >= 0)[0]
        m0[si0[pos] % 128, pos] = 1.0
        m1[si1[pos] % 128, pos] = 1.0
        in_maps.append(
            {
                "m0": m0,
                "m1": m1,
                "embc": np.ascontiguousarray(embc),
                **shared,
            }
        )
    return in_maps


def kernel(x, emb0, emb1, W1, b1, W2, b2, Wg, bg):
    global LAST_EXEC_NS
    cores, slot_pairs, tok_maps, visits, S = assign_slots(x)
    nc = build_program(visits, S)
    in_maps = marshal_inputs(
        x, emb0, emb1, W1, b1, W2, b2, Wg, bg, cores, slot_pairs, S
    )
    trace = os.environ.get("BASSMOE_TRACE", "0") == "1"
    res = run_bass_kernel_spmd(nc, in_maps, list(range(NCORES)), trace=trace)
    LAST_EXEC_NS = res.exec_time_ns
    out = np.empty((B, OUT), dtype=np.float32)
    for c in range(NCORES):
        toks, slots = tok_maps[c]
        r = res.results[c]["out"]  # [128, S]
        out[toks, :] = r[:, slots].T
    return out


# revision 56
# speedup vs baseline: 1.0220x; 1.0009x over previous
"""MoE model via global vocab-pair bucketing + per-core chunk tables on 8 TRN2
cores.

v5 reworks v4's per-core bucketing into a GLOBAL (i0//128, i1//128) pair
bucketing: the host assigns each of the 16 chunk-pair classes to a core (2 per
core, sharing the i0 chunk), so each core precomputes only the 3 vocab-chunk
tables its tokens can touch (T0[c0], T1[c1a], T1[c1b]) instead of all 8 —
cutting the T = emb @ W1 precompute from 278k to ~104k PE cycles — and every
supertile is pure (2 selection matmuls per feature chunk, no mixed spill
tiles).

The main loop is EXPERT-PHASE-MAJOR: phase fb streams W1 block fb (2 x 2 MB),
builds the fb-slice of the 3 chunk tables, then for every supertile does the
8-fc selection + paired silu + expert-fb W2 + gate-combine into a per-tile
fp32 accumulator. Selection work on block 0 therefore overlaps the DMA stream
of blocks 1-3.

Other changes vs v4:
  - one-hot masks come from the host (index marshalling), removing the
    x-broadcast K=1 matmuls and the DVE compares;
  - softmax uses reciprocal_approx_fast (5x faster than DVE reciprocal, which
    stalled the PE ~1us per supertile) and gates are normalized BEFORE the
    per-expert broadcast, dropping the 128-row reciprocal broadcast and the
    final combine multiply;
  - gate-broadcast evac moved from ACT to DVE (ACT is near-saturated by the
    paired silu evacs in the phase loop).
"""

import os
import numpy as np
import ml_dtypes

import concourse.bass as bass
import concourse.mybir as mybir
import concourse.tile as tile
from concourse.bass_utils import run_bass_kernel_spmd

BF16 = ml_dtypes.bfloat16
FP8 = ml_dtypes.float8_e4m3

B = 65536
V = 512
D = 1024
IN = 2048
E = 4
OUT = 128
NCORES = 8
F = E * D                 # 4096 features, expert-major (f = e*1024 + d)
KC = D // 128             # 8 contraction chunks per table
FB = 4                    # W1 feature blocks (1024 feats each == one expert)
NG = V // 128             # 4 vocab chunks per table

LAST_EXEC_NS = None       # set when BASSMOE_TRACE=1


def _legalize_waits(nc, max_waits=1):
    """This walrus build rejects instructions carrying more than ~1 sync-wait
    command; hoist all but the last wait onto single-wait NoOps."""
    for f in nc.m.functions:
        for bb in f.blocks:
            insts = bb.instructions
            if not any(
                inst.sync_info is not None and len(inst.sync_info.on_wait) > max_waits
                for inst in insts
            ):
                continue
            new = []
            for inst in insts:
                si = inst.sync_info
                waits = list(si.on_wait) if si is not None else []
                if len(waits) > max_waits:
                    for w in waits[:-max_waits]:
                        nop = mybir.InstNoOp(
                            name=f"legw-{nc.next_id()}", ins=[], outs=[]
                        )
                        nop.engine = inst.engine
                        nop.sync_info = mybir.SyncInfo(on_wait=[w], on_update=[])
                        new.append(nop)
                    inst.sync_info = mybir.SyncInfo(
                        on_wait=waits[-max_waits:], on_update=list(si.on_update)
                    )
                new.append(inst)
            bb.instructions = new


def build_program(visits, S, legalize=True):
    """visits: list of (offset, width, jk) with jk in {1, 2} naming which T1
    chunk table the supertile's i1 one-hots select from."""
    dt = mybir.dt
    f32, bf16 = dt.float32, dt.bfloat16
    AF = mybir.ActivationFunctionType
    ALU = mybir.AluOpType

    nc = bass.Bass()

    fp8 = dt.float8e4
    # one-hot masks are exact in fp8: halves the startup-critical DMA bytes
    m0d = nc.dram_tensor("m0", [128, S], fp8, kind="ExternalInput")
    m1d = nc.dram_tensor("m1", [128, S], fp8, kind="ExternalInput")
    # embc[p, j, kc, v] = emb_tab(j)[chunk(j)*128 + v, kc*128 + p]
    embtd = nc.dram_tensor("embc", [128, 3, KC, 128], bf16, kind="ExternalInput")
    # w1m[t, fb, p, kc, ff] = W1flat[t*1024 + kc*128 + p, fb*1024 + ff]
    # (partition-major so one coarse dma_start covers a whole block)
    w1d = nc.dram_tensor("w1m", [2, FB, 128, KC, 1024], bf16, kind="ExternalInput")
    b1rd = nc.dram_tensor("b1row", [1, F], bf16, kind="ExternalInput")
    bgrd = nc.dram_tensor("bgrow", [1, E], bf16, kind="ExternalInput")
    wgd = nc.dram_tensor("wgm", [128, 2, KC, E], bf16, kind="ExternalInput")
    w2d = nc.dram_tensor("w2s", [128, E, KC, OUT], bf16, kind="ExternalInput")
    b2d = nc.dram_tensor("b2s", [128, E], f32, kind="ExternalInput")
    seld = nc.dram_tensor("sels", [128, E, 128], bf16, kind="ExternalInput")
    outd = nc.dram_tensor("out", [128, S], f32, kind="ExternalOutput")

    with tile.TileContext(nc) as tc:
        with (
            tc.tile_pool(name="const", bufs=1) as cpool,
            tc.tile_pool(name="w1st", bufs=2) as w1pool,
            tc.tile_pool(name="tt", bufs=2) as tpool,
            tc.tile_pool(name="hs", bufs=2) as hpool,
            tc.tile_pool(name="sm", bufs=2) as smpool,
            tc.tile_pool(name="gate", bufs=1) as gatepool,
            tc.tile_pool(name="accp", bufs=1) as apool,
            tc.tile_pool(name="tmpp", bufs=2) as tmpool,
            tc.tile_pool(name="gsc", bufs=2) as gspool,
            tc.tile_pool(name="pmm", bufs=2, space="PSUM") as pmm,
            tc.tile_pool(name="peo", bufs=1, space="PSUM") as peo,
            tc.tile_pool(name="pmisc", bufs=3, space="PSUM") as pmisc,
        ):
            # --- prologue DMAs, ordered by first use ---
            wg_sb = cpool.tile([128, 2, KC, E], bf16)
            nc.sync.dma_start(wg_sb[:], wgd[:])
            bgr_sb = cpool.tile([1, E], bf16)
            nc.sync.dma_start(bgr_sb[:], bgrd[:])
            # COARSE dma_starts: each one costs ~600ns of Sync-engine issue
            # time, so the startup path uses few large transfers
            embc_sb = cpool.tile([128, 3, KC, 128], bf16)
            nc.sync.dma_start(embc_sb[:, 0], embtd[:, 0])
            w1t0 = w1pool.tile([128, KC, 1024], bf16, tag="w1")
            for k0 in range(0, KC, 2):
                nc.sync.dma_start(
                    w1t0[:, k0 : k0 + 2, :], w1d[0, 0, :, k0 : k0 + 2]
                )
            b1r_sb = cpool.tile([1, F], bf16)
            nc.sync.dma_start(b1r_sb[:], b1rd[:])

            ones128_bf = cpool.tile([1, 128], bf16)
            nc.vector.memset(ones128_bf[:], 1.0)
            # all-ones [128,128]: the sum-exp matmul then lands the sumexp
            # replicated on every output partition (broadcast for free)
            onessq = cpool.tile([128, 128], bf16)
            nc.vector.memset(onessq[:], 1.0)
            # exp'd gating chunk tables, padded to 128 stationary columns
            # (zero cols 4..127) so every main-loop matmul keeps the
            # (128,128) PE tile config — no quadrant-switch drains
            g128_sb = cpool.tile([128, 3, 128], bf16)
            nc.vector.memset(g128_sb[:], 0.0)

            # w1t1 + the remaining embc chunks + first mask pieces, ordered
            # by first use
            m0_sb = cpool.tile([128, S], fp8)
            m1_sb = cpool.tile([128, S], fp8)
            w1t1 = w1pool.tile([128, KC, 1024], bf16, tag="w1")
            nc.sync.dma_start(embc_sb[:, 1], embtd[:, 1])
            nc.sync.dma_start(w1t1[:, 0:4, :], w1d[1, 0, :, 0:4])
            nc.sync.dma_start(w1t1[:, 4:8, :], w1d[1, 0, :, 4:8])
            nc.sync.dma_start(embc_sb[:, 2], embtd[:, 2])
            mp = min(1024, S)
            nc.sync.dma_start(m0_sb[:, 0:mp], m0d[:, 0:mp])
            nc.sync.dma_start(m1_sb[:, 0:mp], m1d[:, 0:mp])

            def emit_g(j):
                # one Exp table chunk; all Exp run before any Silu so the ACT
                # table set loads exactly once each
                tj = 0 if j == 0 else 1
                psg = pmisc.tile([128, E], f32, tag="misc")
                for kc in range(KC):
                    nc.tensor.matmul(
                        psg[:],
                        embc_sb[:, j, kc, :],
                        wg_sb[:, tj, kc, :],
                        start=(kc == 0),
                        stop=(kc == KC - 1 and j != 0),
                    )
                if j == 0:
                    # fold bg into chunk 0's table: psg += ones(v) x bg
                    nc.tensor.matmul(
                        psg[:], ones128_bf[:], bgr_sb[:], start=False, stop=True
                    )
                nc.scalar.activation(g128_sb[:, j, 0:E], psg[:], AF.Exp, bias=0.0)

            # W2 weights beat the mask remainder: the first visit's W2 runs at
            # ~31us while later mask pieces have ~10us of slack per piece
            w2_sb = cpool.tile([128, E, KC, OUT], bf16)
            nc.sync.dma_start(w2_sb[:], w2d[:])
            b2_sb = cpool.tile([128, E], f32)
            nc.sync.dma_start(b2_sb[:], b2d[:])
            sel_sb = cpool.tile([128, E, 128], bf16)
            nc.sync.dma_start(sel_sb[:], seld[:])

            for c0 in range(1024, S, 2048):
                c1 = min(S, c0 + 2048)
                nc.sync.dma_start(m0_sb[:, c0:c1], m0d[:, c0:c1])
                nc.sync.dma_start(m1_sb[:, c0:c1], m1d[:, c0:c1])

            acc = {}
            gates = {}
            gparts = {}

            def emit_gating_part1(vi, off, w, jk):
                # exp-gate selections land on psum rows 0..3 (zero-padded
                # stationary cols keep rows 4..127 at exactly 0)
                psa = pmisc.tile([128, w], f32, tag="misc")
                nc.tensor.matmul(
                    psa[:], g128_sb[:, 0, :], m0_sb[:, off : off + w],
                    start=True, stop=True,
                )
                psb = pmisc.tile([128, w], f32, tag="misc")
                nc.tensor.matmul(
                    psb[:], g128_sb[:, jk, :], m1_sb[:, off : off + w],
                    start=True, stop=True,
                )
                sa = smpool.tile([128, 512], f32, tag="s0")
                nc.vector.tensor_copy(sa[:, 0:w], psa[:])
                expt = smpool.tile([128, 512], bf16, tag="expt")
                nc.vector.tensor_tensor(expt[:, 0:w], sa[:, 0:w], psb[:], ALU.mult)
                gparts[vi] = expt

            def emit_gating_part2a(vi, off, w, jk):
                # sum-exp via all-ones stationary: sumexp appears replicated
                # on every psum row, so 1/sumexp needs no broadcast matmul
                expt = gparts[vi]
                sp = pmisc.tile([128, w], f32, tag="misc")
                nc.tensor.matmul(
                    sp[:], onessq[:], expt[:, 0:w], start=True, stop=True
                )
                rec = smpool.tile([128, 512], f32, tag="rec")
                nc.vector.reciprocal_approx_fast(rec[:, 0:w], sp[:])
                gparts[vi] = (expt, rec)

            def emit_gating_part2b(vi, off, w, jk):
                expt, rec = gparts.pop(vi)
                gt = gatepool.tile([128, 512], bf16, tag=f"gate{vi}")
                nc.vector.tensor_tensor(gt[:, 0:w], expt[:, 0:w], rec[:, 0:w], ALU.mult)
                gates[vi] = gt

            def emit_w2(eop, fb, w, hpair, p):
                for half in range(2):
                    dc = p * 2 + half
                    nc.tensor.matmul(
                        eop[:, 0:w],
                        w2_sb[:, fb, dc, :],
                        hpair[:, half, 0:w],
                        start=(dc == 0),
                        stop=(dc == KC - 1),
                    )

            def emit_combine(p):
                # (eop + b2) * gate -> acc, for the CARRIED visit p
                fb = p["fb"]
                vi, w, off, eop, gs = p["vi"], p["w"], p["off"], p["eop"], p["gs"]
                if fb == 0:
                    a = apool.tile([128, 512], f32, tag=f"acc{vi}")
                    acc[vi] = a
                    nc.vector.scalar_tensor_tensor(
                        a[:, 0:w], eop[:, 0:w], b2_sb[:, fb : fb + 1],
                        gs[:, 0:w], ALU.add, ALU.mult,
                    )
                else:
                    tmp = tmpool.tile([128, 512], f32, tag="tmp")
                    nc.vector.scalar_tensor_tensor(
                        tmp[:, 0:w], eop[:, 0:w], b2_sb[:, fb : fb + 1],
                        gs[:, 0:w], ALU.add, ALU.mult,
                    )
                    nc.vector.tensor_add(acc[vi][:, 0:w], acc[vi][:, 0:w], tmp[:, 0:w])
                if fb == FB - 1:
                    nc.sync.dma_start(outd[:, off : off + w], acc[vi][:, 0:w])

            def flush_pending(p):
                if p is None:
                    return
                emit_w2(p["eop"], p["fb"], p["w"], p["hs"][2], 2)
                emit_w2(p["eop"], p["fb"], p["w"], p["hs"][3], 3)
                emit_combine(p)

            def emit_visit(fb, vi, off, w, jk, t_sb, pending):
                # Software-pipelined across visits: this visit's selection
                # stream carries the PREVIOUS visit's last two W2 pairs and
                # combine, so every W2 matmul trails its silu evac by >2.5us
                # and the PE never waits on the ACT engine.
                if fb == 0:
                    emit_gating_part1(vi, off, w, jk)
                hs = []
                for pair in range(4):
                    hp = pmm.tile([128, 2, 512], f32, tag="mm")
                    # bank-interleaved order (T0h0, T0h1, T1h0, T1h1): every
                    # group-stop is followed by the other bank's stream, so
                    # the stop drain overlaps useful work
                    for half in range(2):
                        lf = pair * 2 + half
                        fs = (lf % 4) * 128
                        nc.tensor.matmul(
                            hp[:, half, 0:w],
                            t_sb[:, 0, lf // 4, fs : fs + 128],
                            m0_sb[:, off : off + w],
                            start=True, stop=False,
                        )
                    for half in range(2):
                        lf = pair * 2 + half
                        fs = (lf % 4) * 128
                        nc.tensor.matmul(
                            hp[:, half, 0:w],
                            t_sb[:, jk, lf // 4, fs : fs + 128],
                            m1_sb[:, off : off + w],
                            start=False, stop=True,
                        )
                    hpair = hpool.tile([128, 2, 512], bf16, tag=f"h{pair}")
                    nc.scalar.activation(
                        hpair[:, :, 0:w], hp[:, :, 0:w], AF.Silu, bias=0.0
                    )
                    hs.append(hpair)
                    if pair == 0 and pending is not None:
                        emit_w2(
                            pending["eop"], pending["fb"], pending["w"],
                            pending["hs"][2], 2,
                        )
                    if pair == 1 and pending is not None:
                        emit_w2(
                            pending["eop"], pending["fb"], pending["w"],
                            pending["hs"][3], 3,
                        )
                        emit_combine(pending)
                    if fb == 0 and pair == 1:
                        emit_gating_part2a(vi, off, w, jk)
                # in-visit W2 pairs 0/1 after the selection: maximum slack for
                # both the silu evacs and the peo buffer release (stt of the
                # carried visit fires ~2us before this point)
                eop = peo.tile([128, 512], f32, tag="eo")
                emit_w2(eop, fb, w, hs[0], 0)
                emit_w2(eop, fb, w, hs[1], 1)
                if fb == 0:
                    emit_gating_part2b(vi, off, w, jk)
                gp = pmisc.tile([128, w], f32, tag="misc")
                nc.tensor.matmul(
                    gp[:], sel_sb[:, fb, :], gates[vi][:, 0:w],
                    start=True, stop=True,
                )
                gs = gspool.tile([128, 512], bf16, tag="gs")
                nc.vector.tensor_copy(gs[:, 0:w], gp[:])
                return dict(vi=vi, off=off, w=w, eop=eop, hs=hs, gs=gs, fb=fb)

            # --- expert-phase-major main loop ---
            # `pending` (deferred W2 tail + combine) carries ACROSS phase
            # boundaries too: the next phase's precompute gives it slack
            pending = None
            for fb in range(FB):
                if fb == 0:
                    t0t, t1t = w1t0, w1t1
                else:
                    t0t = w1pool.tile([128, KC, 1024], bf16, tag="w1")
                    nc.sync.dma_start(t0t[:, 0:4, :], w1d[0, fb, :, 0:4])
                    nc.sync.dma_start(t0t[:, 4:8, :], w1d[0, fb, :, 4:8])
                    t1t = w1pool.tile([128, KC, 1024], bf16, tag="w1")
                    nc.sync.dma_start(t1t[:, 0:4, :], w1d[1, fb, :, 0:4])
                    nc.sync.dma_start(t1t[:, 4:8, :], w1d[1, fb, :, 4:8])
                # build the fb-slice of the 3 chunk tables
                t_sb = tpool.tile([128, 3, 2, 512], bf16, tag="t")

                def emit_tchunk(fb, j, w1t, t_sb):
                    ps = pmm.tile([128, 2, 512], f32, tag="mm")
                    for fp in range(2):
                        for kc in range(KC):
                            nc.tensor.matmul(
                                ps[:, fp, :],
                                embc_sb[:, j, kc, :],
                                w1t[:, kc, fp * 512 : (fp + 1) * 512],
                                start=(kc == 0),
                                stop=(kc == KC - 1 and j != 0),
                            )
                        if j == 0:
                            f0 = fb * 1024 + fp * 512
                            nc.tensor.matmul(
                                ps[:, fp, :], ones128_bf[:],
                                b1r_sb[:, f0 : f0 + 512],
                                start=False, stop=True,
                            )
                    nc.scalar.copy(t_sb[:, j, :, :], ps[:])

                # bucket-b work (chunk-2 table, G2) is deferred past the
                # bucket-a visits: it is not needed until the first jk==2
                # visit, and in phase 0 this shortens the DMA-bound startup
                if fb == 0:
                    # G0 first: it needs only wg + embc chunk 0 (~0.3MB) and
                    # fills the PE while the first w1 block is still landing
                    emit_g(0)
                    emit_tchunk(fb, 0, t0t, t_sb)
                    emit_tchunk(fb, 1, t1t, t_sb)
                    emit_g(1)
                    emit_tchunk(fb, 2, t1t, t_sb)
                    emit_g(2)
                else:
                    for j in range(3):
                        emit_tchunk(fb, j, t0t if j == 0 else t1t, t_sb)
                for vi, (off, w, jk) in enumerate(visits):
                    pending = emit_visit(fb, vi, off, w, jk, t_sb, pending)
            flush_pending(pending)

    if legalize:
        _legalize_waits(nc)
    mybir.codegen_inst_isa_subclasses(nc)
    return nc


def _roundup(n, m):
    return -(-n // m) * m


def assign_slots(x):
    """Global (i0//128, i1//128) bucketing: 16 classes -> 8 cores (2 each,
    sharing the i0 chunk), with identical (i0, i1) token pairs DEDUPED —
    the device computes each distinct pair once and the host scatters the
    result to every duplicate token (~11% fewer slots on uniform data)."""
    x = np.asarray(x)
    c0 = x[:, 0] // 128
    c1 = x[:, 1] // 128
    key = x[:, 0] * V + x[:, 1]
    cores = []
    for g in range(NG):
        cls = []
        for b in range(NG):
            idx = np.nonzero((c0 == g) & (c1 == b))[0]
            ukey, inv = np.unique(key[idx], return_inverse=True)
            cls.append((idx, ukey, inv))
        order = sorted(range(NG), key=lambda b: -len(cls[b][1]))
        for pa, pb in ((order[0], order[3]), (order[1], order[2])):
            if len(cls[pb][1]) > len(cls[pa][1]):
                pa, pb = pb, pa
            cores.append(dict(c0=g, c1a=pa, c1b=pb, a=cls[pa], b=cls[pb]))
    SA = _roundup(max(len(c["a"][1]) for c in cores), 64)
    SB = _roundup(max(len(c["b"][1]) for c in cores), 64)
    visits = []
    off = 0
    for span, jk in ((SA, 1), (SB, 2)):
        left = span
        while left > 0:
            w = min(512, left)
            visits.append((off, w, jk))
            off += w
            left -= w
    S = SA + SB
    slot_pairs = []   # per core: slot -> (i0, i1), -1 for pad
    tok_maps = []     # per core: (global token ids, their slots)
    for c in cores:
        si0 = np.full(S, -1, dtype=np.int64)
        si1 = np.full(S, -1, dtype=np.int64)
        (ia, ka, va), (ib, kb, vb) = c["a"], c["b"]
        si0[0 : len(ka)] = ka // V
        si1[0 : len(ka)] = ka % V
        si0[SA : SA + len(kb)] = kb // V
        si1[SA : SA + len(kb)] = kb % V
        slot_pairs.append((si0, si1))
        tok_maps.append(
            (np.concatenate([ia, ib]), np.concatenate([va, SA + vb]))
        )
    return cores, slot_pairs, tok_maps, visits, S


def marshal_inputs(x, emb0, emb1, W1, b1, W2, b2, Wg, bg, cores, slot_pairs, S):
    x = np.asarray(x)
    emb0 = np.asarray(emb0)
    emb1 = np.asarray(emb1)

    shared = {}
    # W1flat[k, f] with f = e*1024 + d (expert-major features)
    w1flat = np.asarray(W1).transpose(1, 0, 2).reshape(IN, F)
    shared["w1m"] = np.ascontiguousarray(
        w1flat.reshape(2, KC, 128, FB, 1024).transpose(0, 3, 2, 1, 4).astype(BF16)
    )
    shared["b1row"] = np.ascontiguousarray(
        np.asarray(b1).reshape(1, F).astype(BF16)
    )
    shared["bgrow"] = np.ascontiguousarray(
        np.asarray(bg).reshape(1, E).astype(BF16)
    )
    shared["wgm"] = np.ascontiguousarray(
        np.asarray(Wg).reshape(2, KC, 128, E).transpose(2, 0, 1, 3).astype(BF16)
    )
    shared["w2s"] = np.ascontiguousarray(
        np.asarray(W2).reshape(E, KC, 128, OUT).transpose(2, 0, 1, 3).astype(BF16)
    )
    shared["b2s"] = np.ascontiguousarray(np.asarray(b2).T.astype(np.float32))
    # sel128[p, e, o] = 1 iff p == e: a (128,128)-tile gate-row broadcast
    sel128 = np.zeros((128, E, 128), dtype=BF16)
    for e in range(E):
        sel128[e, e, :] = 1.0
    shared["sels"] = np.ascontiguousarray(sel128)

    in_maps = []
    for c, (si0, si1) in zip(cores, slot_pairs):
        # embc[p, j, kc, v] = emb_tab(j)[chunk(j)*128 + v, kc*128 + p]
        embc = np.empty((128, 3, KC, 128), dtype=BF16)
        for j, (tab, ch) in enumerate(
            ((emb0, c["c0"]), (emb1, c["c1a"]), (emb1, c["c1b"]))
        ):
            chunk = tab[ch * 128 : (ch + 1) * 128]  # [128v, 1024k]
            embc[:, j] = (
                chunk.reshape(128, KC, 128).transpose(2, 1, 0).astype(BF16)
            )
        # one-hot masks per slot (pad slots stay all-zero); fp8 is exact
        m0 = np.zeros((128, S), dtype=FP8)
        m1 = np.zeros((128, S), dtype=FP8)
        pos = np.nonzero(si0 >= 0)[0]
        m0[si0[pos] % 128, pos] = 1.0
        m1[si1[pos] % 128, pos] = 1.0
        in_maps.append(
            {
                "m0": m0,
                "m1": m1,
                "embc": np.ascontiguousarray(embc),
                **shared,
            }
        )
    return in_maps


def kernel(x, emb0, emb1, W1, b1, W2, b2, Wg, bg):
    global LAST_EXEC_NS
    cores, slot_pairs, tok_maps, visits, S = assign_slots(x)
    nc = build_program(visits, S)
    in_maps = marshal_inputs(
        x, emb0, emb1, W1, b1, W2, b2, Wg, bg, cores, slot_pairs, S
    )
    trace = os.environ.get("BASSMOE_TRACE", "0") == "1"
    res = run_bass_kernel_spmd(nc, in_maps, list(range(NCORES)), trace=trace)
    LAST_EXEC_NS = res.exec_time_ns
    out = np.empty((B, OUT), dtype=np.float32)
    for c in range(NCORES):
        toks, slots = tok_maps[c]
        r = res.results[c]["out"]  # [128, S]
        out[toks, :] = r[:, slots].T
    return out


# revision 57
# speedup vs baseline: 1.0238x; 1.0017x over previous
"""MoE model via global vocab-pair bucketing + per-core chunk tables on 8 TRN2
cores.

v5 reworks v4's per-core bucketing into a GLOBAL (i0//128, i1//128) pair
bucketing: the host assigns each of the 16 chunk-pair classes to a core (2 per
core, sharing the i0 chunk), so each core precomputes only the 3 vocab-chunk
tables its tokens can touch (T0[c0], T1[c1a], T1[c1b]) instead of all 8 —
cutting the T = emb @ W1 precompute from 278k to ~104k PE cycles — and every
supertile is pure (2 selection matmuls per feature chunk, no mixed spill
tiles).

The main loop is EXPERT-PHASE-MAJOR: phase fb streams W1 block fb (2 x 2 MB),
builds the fb-slice of the 3 chunk tables, then for every supertile does the
8-fc selection + paired silu + expert-fb W2 + gate-combine into a per-tile
fp32 accumulator. Selection work on block 0 therefore overlaps the DMA stream
of blocks 1-3.

Other changes vs v4:
  - one-hot masks come from the host (index marshalling), removing the
    x-broadcast K=1 matmuls and the DVE compares;
  - softmax uses reciprocal_approx_fast (5x faster than DVE reciprocal, which
    stalled the PE ~1us per supertile) and gates are normalized BEFORE the
    per-expert broadcast, dropping the 128-row reciprocal broadcast and the
    final combine multiply;
  - gate-broadcast evac moved from ACT to DVE (ACT is near-saturated by the
    paired silu evacs in the phase loop).
"""

import os
import numpy as np
import ml_dtypes

import concourse.bass as bass
import concourse.mybir as mybir
import concourse.tile as tile
from concourse.bass_utils import run_bass_kernel_spmd

BF16 = ml_dtypes.bfloat16
FP8 = ml_dtypes.float8_e4m3

B = 65536
V = 512
D = 1024
IN = 2048
E = 4
OUT = 128
NCORES = 8
F = E * D                 # 4096 features, expert-major (f = e*1024 + d)
KC = D // 128             # 8 contraction chunks per table
FB = 4                    # W1 feature blocks (1024 feats each == one expert)
NG = V // 128             # 4 vocab chunks per table

LAST_EXEC_NS = None       # set when BASSMOE_TRACE=1


def _legalize_waits(nc, max_waits=1):
    """This walrus build rejects instructions carrying more than ~1 sync-wait
    command; hoist all but the last wait onto single-wait NoOps."""
    for f in nc.m.functions:
        for bb in f.blocks:
            insts = bb.instructions
            if not any(
                inst.sync_info is not None and len(inst.sync_info.on_wait) > max_waits
                for inst in insts
            ):
                continue
            new = []
            for inst in insts:
                si = inst.sync_info
                waits = list(si.on_wait) if si is not None else []
                if len(waits) > max_waits:
                    for w in waits[:-max_waits]:
                        nop = mybir.InstNoOp(
                            name=f"legw-{nc.next_id()}", ins=[], outs=[]
                        )
                        nop.engine = inst.engine
                        nop.sync_info = mybir.SyncInfo(on_wait=[w], on_update=[])
                        new.append(nop)
                    inst.sync_info = mybir.SyncInfo(
                        on_wait=waits[-max_waits:], on_update=list(si.on_update)
                    )
                new.append(inst)
            bb.instructions = new


def build_program(visits, S, legalize=True):
    """visits: list of (offset, width, jk) with jk in {1, 2} naming which T1
    chunk table the supertile's i1 one-hots select from."""
    dt = mybir.dt
    f32, bf16 = dt.float32, dt.bfloat16
    AF = mybir.ActivationFunctionType
    ALU = mybir.AluOpType

    nc = bass.Bass()

    fp8 = dt.float8e4
    # one-hot masks are exact in fp8: halves the startup-critical DMA bytes
    m0d = nc.dram_tensor("m0", [128, S], fp8, kind="ExternalInput")
    m1d = nc.dram_tensor("m1", [128, S], fp8, kind="ExternalInput")
    # embc[p, j, kc, v] = emb_tab(j)[chunk(j)*128 + v, kc*128 + p]
    embtd = nc.dram_tensor("embc", [128, 3, KC, 128], bf16, kind="ExternalInput")
    # w1m[t, fb, p, kc, ff] = W1flat[t*1024 + kc*128 + p, fb*1024 + ff]
    # (partition-major so one coarse dma_start covers a whole block)
    w1d = nc.dram_tensor("w1m", [2, FB, 128, KC, 1024], bf16, kind="ExternalInput")
    b1rd = nc.dram_tensor("b1row", [1, F], bf16, kind="ExternalInput")
    bgrd = nc.dram_tensor("bgrow", [1, E], bf16, kind="ExternalInput")
    wgd = nc.dram_tensor("wgm", [128, 2, KC, E], bf16, kind="ExternalInput")
    w2d = nc.dram_tensor("w2s", [128, E, KC, OUT], bf16, kind="ExternalInput")
    b2d = nc.dram_tensor("b2s", [128, E], f32, kind="ExternalInput")
    seld = nc.dram_tensor("sels", [128, E, 128], bf16, kind="ExternalInput")
    outd = nc.dram_tensor("out", [128, S], f32, kind="ExternalOutput")

    with tile.TileContext(nc) as tc:
        with (
            tc.tile_pool(name="const", bufs=1) as cpool,
            tc.tile_pool(name="w1st", bufs=2) as w1pool,
            tc.tile_pool(name="tt", bufs=2) as tpool,
            tc.tile_pool(name="hs", bufs=2) as hpool,
            tc.tile_pool(name="sm", bufs=2) as smpool,
            tc.tile_pool(name="gate", bufs=1) as gatepool,
            tc.tile_pool(name="accp", bufs=1) as apool,
            tc.tile_pool(name="tmpp", bufs=2) as tmpool,
            tc.tile_pool(name="gsc", bufs=2) as gspool,
            tc.tile_pool(name="pmm", bufs=2, space="PSUM") as pmm,
            tc.tile_pool(name="peo", bufs=1, space="PSUM") as peo,
            tc.tile_pool(name="pmisc", bufs=3, space="PSUM") as pmisc,
        ):
            # --- prologue DMAs, ordered by first use ---
            # COARSE dma_starts: each one costs ~600ns of Sync-engine issue
            # time, so the startup path uses few large transfers. The first
            # w1 piece + embc chunk 0 lead: they gate the first T matmuls.
            embc_sb = cpool.tile([128, 3, KC, 128], bf16)
            w1t0 = w1pool.tile([128, KC, 1024], bf16, tag="w1")
            nc.sync.dma_start(w1t0[:, 0:2, :], w1d[0, 0, :, 0:2])
            nc.sync.dma_start(embc_sb[:, 0], embtd[:, 0])
            wg_sb = cpool.tile([128, 2, KC, E], bf16)
            nc.sync.dma_start(wg_sb[:], wgd[:])
            bgr_sb = cpool.tile([1, E], bf16)
            nc.sync.dma_start(bgr_sb[:], bgrd[:])
            for k0 in range(2, KC, 2):
                nc.sync.dma_start(
                    w1t0[:, k0 : k0 + 2, :], w1d[0, 0, :, k0 : k0 + 2]
                )
            b1r_sb = cpool.tile([1, F], bf16)
            nc.sync.dma_start(b1r_sb[:], b1rd[:])

            ones128_bf = cpool.tile([1, 128], bf16)
            nc.vector.memset(ones128_bf[:], 1.0)
            # all-ones [128,128]: the sum-exp matmul then lands the sumexp
            # replicated on every output partition (broadcast for free)
            onessq = cpool.tile([128, 128], bf16)
            nc.vector.memset(onessq[:], 1.0)
            # exp'd gating chunk tables, padded to 128 stationary columns
            # (zero cols 4..127) so every main-loop matmul keeps the
            # (128,128) PE tile config — no quadrant-switch drains
            g128_sb = cpool.tile([128, 3, 128], bf16)
            nc.vector.memset(g128_sb[:], 0.0)

            # w1t1 + the remaining embc chunks + first mask pieces, ordered
            # by first use
            m0_sb = cpool.tile([128, S], fp8)
            m1_sb = cpool.tile([128, S], fp8)
            w1t1 = w1pool.tile([128, KC, 1024], bf16, tag="w1")
            nc.sync.dma_start(embc_sb[:, 1], embtd[:, 1])
            nc.sync.dma_start(w1t1[:, 0:4, :], w1d[1, 0, :, 0:4])
            nc.sync.dma_start(w1t1[:, 4:8, :], w1d[1, 0, :, 4:8])
            nc.sync.dma_start(embc_sb[:, 2], embtd[:, 2])
            mp = min(1024, S)
            nc.sync.dma_start(m0_sb[:, 0:mp], m0d[:, 0:mp])
            nc.sync.dma_start(m1_sb[:, 0:mp], m1d[:, 0:mp])

            def emit_g(j):
                # one Exp table chunk; all Exp run before any Silu so the ACT
                # table set loads exactly once each
                tj = 0 if j == 0 else 1
                psg = pmisc.tile([128, E], f32, tag="misc")
                for kc in range(KC):
                    nc.tensor.matmul(
                        psg[:],
                        embc_sb[:, j, kc, :],
                        wg_sb[:, tj, kc, :],
                        start=(kc == 0),
                        stop=(kc == KC - 1 and j != 0),
                    )
                if j == 0:
                    # fold bg into chunk 0's table: psg += ones(v) x bg
                    nc.tensor.matmul(
                        psg[:], ones128_bf[:], bgr_sb[:], start=False, stop=True
                    )
                nc.scalar.activation(g128_sb[:, j, 0:E], psg[:], AF.Exp, bias=0.0)

            # W2 weights beat the mask remainder: the first visit's W2 runs at
            # ~31us while later mask pieces have ~10us of slack per piece
            w2_sb = cpool.tile([128, E, KC, OUT], bf16)
            nc.sync.dma_start(w2_sb[:], w2d[:])
            b2_sb = cpool.tile([128, E], f32)
            nc.sync.dma_start(b2_sb[:], b2d[:])
            sel_sb = cpool.tile([128, E, 128], bf16)
            nc.sync.dma_start(sel_sb[:], seld[:])

            for c0 in range(1024, S, 2048):
                c1 = min(S, c0 + 2048)
                nc.sync.dma_start(m0_sb[:, c0:c1], m0d[:, c0:c1])
                nc.sync.dma_start(m1_sb[:, c0:c1], m1d[:, c0:c1])

            acc = {}
            gates = {}
            gparts = {}

            def emit_gating_part1(vi, off, w, jk):
                # exp-gate selections land on psum rows 0..3 (zero-padded
                # stationary cols keep rows 4..127 at exactly 0)
                psa = pmisc.tile([128, w], f32, tag="misc")
                nc.tensor.matmul(
                    psa[:], g128_sb[:, 0, :], m0_sb[:, off : off + w],
                    start=True, stop=True,
                )
                psb = pmisc.tile([128, w], f32, tag="misc")
                nc.tensor.matmul(
                    psb[:], g128_sb[:, jk, :], m1_sb[:, off : off + w],
                    start=True, stop=True,
                )
                sa = smpool.tile([128, 512], f32, tag="s0")
                nc.vector.tensor_copy(sa[:, 0:w], psa[:])
                expt = smpool.tile([128, 512], bf16, tag="expt")
                nc.vector.tensor_tensor(expt[:, 0:w], sa[:, 0:w], psb[:], ALU.mult)
                gparts[vi] = expt

            def emit_gating_part2a(vi, off, w, jk):
                # sum-exp via all-ones stationary: sumexp appears replicated
                # on every psum row, so 1/sumexp needs no broadcast matmul
                expt = gparts[vi]
                sp = pmisc.tile([128, w], f32, tag="misc")
                nc.tensor.matmul(
                    sp[:], onessq[:], expt[:, 0:w], start=True, stop=True
                )
                rec = smpool.tile([128, 512], f32, tag="rec")
                nc.vector.reciprocal_approx_fast(rec[:, 0:w], sp[:])
                gparts[vi] = (expt, rec)

            def emit_gating_part2b(vi, off, w, jk):
                expt, rec = gparts.pop(vi)
                gt = gatepool.tile([128, 512], bf16, tag=f"gate{vi}")
                nc.vector.tensor_tensor(gt[:, 0:w], expt[:, 0:w], rec[:, 0:w], ALU.mult)
                gates[vi] = gt

            def emit_w2(eop, fb, w, hpair, p):
                for half in range(2):
                    dc = p * 2 + half
                    nc.tensor.matmul(
                        eop[:, 0:w],
                        w2_sb[:, fb, dc, :],
                        hpair[:, half, 0:w],
                        start=(dc == 0),
                        stop=(dc == KC - 1),
                    )

            def emit_combine(p):
                # (eop + b2) * gate -> acc, for the CARRIED visit p
                fb = p["fb"]
                vi, w, off, eop, gs = p["vi"], p["w"], p["off"], p["eop"], p["gs"]
                if fb == 0:
                    a = apool.tile([128, 512], f32, tag=f"acc{vi}")
                    acc[vi] = a
                    nc.vector.scalar_tensor_tensor(
                        a[:, 0:w], eop[:, 0:w], b2_sb[:, fb : fb + 1],
                        gs[:, 0:w], ALU.add, ALU.mult,
                    )
                else:
                    tmp = tmpool.tile([128, 512], f32, tag="tmp")
                    nc.vector.scalar_tensor_tensor(
                        tmp[:, 0:w], eop[:, 0:w], b2_sb[:, fb : fb + 1],
                        gs[:, 0:w], ALU.add, ALU.mult,
                    )
                    nc.vector.tensor_add(acc[vi][:, 0:w], acc[vi][:, 0:w], tmp[:, 0:w])
                if fb == FB - 1:
                    nc.sync.dma_start(outd[:, off : off + w], acc[vi][:, 0:w])

            def flush_pending(p):
                if p is None:
                    return
                emit_w2(p["eop"], p["fb"], p["w"], p["hs"][2], 2)
                emit_w2(p["eop"], p["fb"], p["w"], p["hs"][3], 3)
                emit_combine(p)

            def emit_visit(fb, vi, off, w, jk, t_sb, pending):
                # Software-pipelined across visits: this visit's selection
                # stream carries the PREVIOUS visit's last two W2 pairs and
                # combine, so every W2 matmul trails its silu evac by >2.5us
                # and the PE never waits on the ACT engine.
                if fb == 0:
                    emit_gating_part1(vi, off, w, jk)
                hs = []
                for pair in range(4):
                    hp = pmm.tile([128, 2, 512], f32, tag="mm")
                    # bank-interleaved order (T0h0, T0h1, T1h0, T1h1): every
                    # group-stop is followed by the other bank's stream, so
                    # the stop drain overlaps useful work
                    for half in range(2):
                        lf = pair * 2 + half
                        fs = (lf % 4) * 128
                        nc.tensor.matmul(
                            hp[:, half, 0:w],
                            t_sb[:, 0, lf // 4, fs : fs + 128],
                            m0_sb[:, off : off + w],
                            start=True, stop=False,
                        )
                    for half in range(2):
                        lf = pair * 2 + half
                        fs = (lf % 4) * 128
                        nc.tensor.matmul(
                            hp[:, half, 0:w],
                            t_sb[:, jk, lf // 4, fs : fs + 128],
                            m1_sb[:, off : off + w],
                            start=False, stop=True,
                        )
                    hpair = hpool.tile([128, 2, 512], bf16, tag=f"h{pair}")
                    nc.scalar.activation(
                        hpair[:, :, 0:w], hp[:, :, 0:w], AF.Silu, bias=0.0
                    )
                    hs.append(hpair)
                    if pair == 0 and pending is not None:
                        emit_w2(
                            pending["eop"], pending["fb"], pending["w"],
                            pending["hs"][2], 2,
                        )
                    if pair == 1 and pending is not None:
                        emit_w2(
                            pending["eop"], pending["fb"], pending["w"],
                            pending["hs"][3], 3,
                        )
                        emit_combine(pending)
                    if fb == 0 and pair == 1:
                        emit_gating_part2a(vi, off, w, jk)
                # in-visit W2 pairs 0/1 after the selection: maximum slack for
                # both the silu evacs and the peo buffer release (stt of the
                # carried visit fires ~2us before this point)
                eop = peo.tile([128, 512], f32, tag="eo")
                emit_w2(eop, fb, w, hs[0], 0)
                emit_w2(eop, fb, w, hs[1], 1)
                if fb == 0:
                    emit_gating_part2b(vi, off, w, jk)
                gp = pmisc.tile([128, w], f32, tag="misc")
                nc.tensor.matmul(
                    gp[:], sel_sb[:, fb, :], gates[vi][:, 0:w],
                    start=True, stop=True,
                )
                gs = gspool.tile([128, 512], bf16, tag="gs")
                nc.vector.tensor_copy(gs[:, 0:w], gp[:])
                return dict(vi=vi, off=off, w=w, eop=eop, hs=hs, gs=gs, fb=fb)

            # --- expert-phase-major main loop ---
            # `pending` (deferred W2 tail + combine) carries ACROSS phase
            # boundaries too: the next phase's precompute gives it slack
            pending = None
            for fb in range(FB):
                if fb == 0:
                    t0t, t1t = w1t0, w1t1
                else:
                    t0t = w1pool.tile([128, KC, 1024], bf16, tag="w1")
                    nc.sync.dma_start(t0t[:, 0:4, :], w1d[0, fb, :, 0:4])
                    nc.sync.dma_start(t0t[:, 4:8, :], w1d[0, fb, :, 4:8])
                    t1t = w1pool.tile([128, KC, 1024], bf16, tag="w1")
                    nc.sync.dma_start(t1t[:, 0:4, :], w1d[1, fb, :, 0:4])
                    nc.sync.dma_start(t1t[:, 4:8, :], w1d[1, fb, :, 4:8])
                # build the fb-slice of the 3 chunk tables
                t_sb = tpool.tile([128, 3, 2, 512], bf16, tag="t")

                def emit_tchunk(fb, j, w1t, t_sb):
                    ps = pmm.tile([128, 2, 512], f32, tag="mm")
                    for fp in range(2):
                        for kc in range(KC):
                            nc.tensor.matmul(
                                ps[:, fp, :],
                                embc_sb[:, j, kc, :],
                                w1t[:, kc, fp * 512 : (fp + 1) * 512],
                                start=(kc == 0),
                                stop=(kc == KC - 1 and j != 0),
                            )
                        if j == 0:
                            f0 = fb * 1024 + fp * 512
                            nc.tensor.matmul(
                                ps[:, fp, :], ones128_bf[:],
                                b1r_sb[:, f0 : f0 + 512],
                                start=False, stop=True,
                            )
                    nc.scalar.copy(t_sb[:, j, :, :], ps[:])

                # bucket-b work (chunk-2 table, G2) is deferred past the
                # bucket-a visits: it is not needed until the first jk==2
                # visit, and in phase 0 this shortens the DMA-bound startup
                if fb == 0:
                    # G0 first: it needs only wg + embc chunk 0 (~0.3MB) and
                    # fills the PE while the first w1 block is still landing
                    emit_g(0)
                    emit_tchunk(fb, 0, t0t, t_sb)
                    emit_tchunk(fb, 1, t1t, t_sb)
                    emit_g(1)
                    emit_tchunk(fb, 2, t1t, t_sb)
                    emit_g(2)
                else:
                    for j in range(3):
                        emit_tchunk(fb, j, t0t if j == 0 else t1t, t_sb)
                for vi, (off, w, jk) in enumerate(visits):
                    pending = emit_visit(fb, vi, off, w, jk, t_sb, pending)
            flush_pending(pending)

    if legalize:
        _legalize_waits(nc)
    mybir.codegen_inst_isa_subclasses(nc)
    return nc


def _roundup(n, m):
    return -(-n // m) * m


def assign_slots(x):
    """Global (i0//128, i1//128) bucketing: 16 classes -> 8 cores (2 each,
    sharing the i0 chunk), with identical (i0, i1) token pairs DEDUPED —
    the device computes each distinct pair once and the host scatters the
    result to every duplicate token (~11% fewer slots on uniform data)."""
    x = np.asarray(x)
    c0 = x[:, 0] // 128
    c1 = x[:, 1] // 128
    key = x[:, 0] * V + x[:, 1]
    cores = []
    for g in range(NG):
        cls = []
        for b in range(NG):
            idx = np.nonzero((c0 == g) & (c1 == b))[0]
            ukey, inv = np.unique(key[idx], return_inverse=True)
            cls.append((idx, ukey, inv))
        order = sorted(range(NG), key=lambda b: -len(cls[b][1]))
        for pa, pb in ((order[0], order[3]), (order[1], order[2])):
            if len(cls[pb][1]) > len(cls[pa][1]):
                pa, pb = pb, pa
            cores.append(dict(c0=g, c1a=pa, c1b=pb, a=cls[pa], b=cls[pb]))
    SA = _roundup(max(len(c["a"][1]) for c in cores), 64)
    SB = _roundup(max(len(c["b"][1]) for c in cores), 64)
    visits = []
    off = 0
    for span, jk in ((SA, 1), (SB, 2)):
        left = span
        while left > 0:
            w = min(512, left)
            visits.append((off, w, jk))
            off += w
            left -= w
    S = SA + SB
    slot_pairs = []   # per core: slot -> (i0, i1), -1 for pad
    tok_maps = []     # per core: (global token ids, their slots)
    for c in cores:
        si0 = np.full(S, -1, dtype=np.int64)
        si1 = np.full(S, -1, dtype=np.int64)
        (ia, ka, va), (ib, kb, vb) = c["a"], c["b"]
        si0[0 : len(ka)] = ka // V
        si1[0 : len(ka)] = ka % V
        si0[SA : SA + len(kb)] = kb // V
        si1[SA : SA + len(kb)] = kb % V
        slot_pairs.append((si0, si1))
        tok_maps.append(
            (np.concatenate([ia, ib]), np.concatenate([va, SA + vb]))
        )
    return cores, slot_pairs, tok_maps, visits, S


def marshal_inputs(x, emb0, emb1, W1, b1, W2, b2, Wg, bg, cores, slot_pairs, S):
    x = np.asarray(x)
    emb0 = np.asarray(emb0)
    emb1 = np.asarray(emb1)

    shared = {}
    # W1flat[k, f] with f = e*1024 + d (expert-major features)
    w1flat = np.asarray(W1).transpose(1, 0, 2).reshape(IN, F)
    shared["w1m"] = np.ascontiguousarray(
        w1flat.reshape(2, KC, 128, FB, 1024).transpose(0, 3, 2, 1, 4).astype(BF16)
    )
    shared["b1row"] = np.ascontiguousarray(
        np.asarray(b1).reshape(1, F).astype(BF16)
    )
    shared["bgrow"] = np.ascontiguousarray(
        np.asarray(bg).reshape(1, E).astype(BF16)
    )
    shared["wgm"] = np.ascontiguousarray(
        np.asarray(Wg).reshape(2, KC, 128, E).transpose(2, 0, 1, 3).astype(BF16)
    )
    shared["w2s"] = np.ascontiguousarray(
        np.asarray(W2).reshape(E, KC, 128, OUT).transpose(2, 0, 1, 3).astype(BF16)
    )
    shared["b2s"] = np.ascontiguousarray(np.asarray(b2).T.astype(np.float32))
    # sel128[p, e, o] = 1 iff p == e: a (128,128)-tile gate-row broadcast
    sel128 = np.zeros((128, E, 128), dtype=BF16)
    for e in range(E):
        sel128[e, e, :] = 1.0
    shared["sels"] = np.ascontiguousarray(sel128)

    in_maps = []
    for c, (si0, si1) in zip(cores, slot_pairs):
        # embc[p, j, kc, v] = emb_tab(j)[chunk(j)*128 + v, kc*128 + p]
        embc = np.empty((128, 3, KC, 128), dtype=BF16)
        for j, (tab, ch) in enumerate(
            ((emb0, c["c0"]), (emb1, c["c1a"]), (emb1, c["c1b"]))
        ):
            chunk = tab[ch * 128 : (ch + 1) * 128]  # [128v, 1024k]
            embc[:, j] = (
                chunk.reshape(128, KC, 128).transpose(2, 1, 0).astype(BF16)
            )
        # one-hot masks per slot (pad slots stay all-zero); fp8 is exact
        m0 = np.zeros((128, S), dtype=FP8)
        m1 = np.zeros((128, S), dtype=FP8)
        pos = np.nonzero(si0 >= 0)[0]
        m0[si0[pos] % 128, pos] = 1.0
        m1[si1[pos] % 128, pos] = 1.0
        in_maps.append(
            {
                "m0": m0,
                "m1": m1,
                "embc": np.ascontiguousarray(embc),
                **shared,
            }
        )
    return in_maps


def kernel(x, emb0, emb1, W1, b1, W2, b2, Wg, bg):
    global LAST_EXEC_NS
    cores, slot_pairs, tok_maps, visits, S = assign_slots(x)
    nc = build_program(visits, S)
    in_maps = marshal_inputs(
        x, emb0, emb1, W1, b1, W2, b2, Wg, bg, cores, slot_pairs, S
    )
    trace = os.environ.get("BASSMOE_TRACE", "0") == "1"
    res = run_bass_kernel_spmd(nc, in_maps, list(range(NCORES)), trace=trace)
    LAST_EXEC_NS = res.exec_time_ns
    out = np.empty((B, OUT), dtype=np.float32)
    for c in range(NCORES):
        toks, slots = tok_maps[c]
        r = res.results[c]["out"]  # [128, S]
        out[toks, :] = r[:, slots].T
    return out


# revision 62
# speedup vs baseline: 1.0268x; 1.0029x over previous
"""MoE model via global vocab-pair bucketing + per-core chunk tables on 8 TRN2
cores.

v5 reworks v4's per-core bucketing into a GLOBAL (i0//128, i1//128) pair
bucketing: the host assigns each of the 16 chunk-pair classes to a core (2 per
core, sharing the i0 chunk), so each core precomputes only the 3 vocab-chunk
tables its tokens can touch (T0[c0], T1[c1a], T1[c1b]) instead of all 8 —
cutting the T = emb @ W1 precompute from 278k to ~104k PE cycles — and every
supertile is pure (2 selection matmuls per feature chunk, no mixed spill
tiles).

The main loop is EXPERT-PHASE-MAJOR: phase fb streams W1 block fb (2 x 2 MB),
builds the fb-slice of the 3 chunk tables, then for every supertile does the
8-fc selection + paired silu + expert-fb W2 + gate-combine into a per-tile
fp32 accumulator. Selection work on block 0 therefore overlaps the DMA stream
of blocks 1-3.

Other changes vs v4:
  - one-hot masks come from the host (index marshalling), removing the
    x-broadcast K=1 matmuls and the DVE compares;
  - softmax uses reciprocal_approx_fast (5x faster than DVE reciprocal, which
    stalled the PE ~1us per supertile) and gates are normalized BEFORE the
    per-expert broadcast, dropping the 128-row reciprocal broadcast and the
    final combine multiply;
  - gate-broadcast evac moved from ACT to DVE (ACT is near-saturated by the
    paired silu evacs in the phase loop).
"""

import os
import numpy as np
import ml_dtypes

import concourse.bass as bass
import concourse.mybir as mybir
import concourse.tile as tile
from concourse.bass_utils import run_bass_kernel_spmd

BF16 = ml_dtypes.bfloat16
FP8 = ml_dtypes.float8_e4m3

B = 65536
V = 512
D = 1024
IN = 2048
E = 4
OUT = 128
NCORES = 8
F = E * D                 # 4096 features, expert-major (f = e*1024 + d)
KC = D // 128             # 8 contraction chunks per table
FB = 4                    # W1 feature blocks (1024 feats each == one expert)
NG = V // 128             # 4 vocab chunks per table

LAST_EXEC_NS = None       # set when BASSMOE_TRACE=1


def _legalize_waits(nc, max_waits=1):
    """This walrus build rejects instructions carrying more than ~1 sync-wait
    command; hoist all but the last wait onto single-wait NoOps."""
    for f in nc.m.functions:
        for bb in f.blocks:
            insts = bb.instructions
            if not any(
                inst.sync_info is not None and len(inst.sync_info.on_wait) > max_waits
                for inst in insts
            ):
                continue
            new = []
            for inst in insts:
                si = inst.sync_info
                waits = list(si.on_wait) if si is not None else []
                if len(waits) > max_waits:
                    for w in waits[:-max_waits]:
                        nop = mybir.InstNoOp(
                            name=f"legw-{nc.next_id()}", ins=[], outs=[]
                        )
                        nop.engine = inst.engine
                        nop.sync_info = mybir.SyncInfo(on_wait=[w], on_update=[])
                        new.append(nop)
                    inst.sync_info = mybir.SyncInfo(
                        on_wait=waits[-max_waits:], on_update=list(si.on_update)
                    )
                new.append(inst)
            bb.instructions = new


def build_program(visits, S, use_b1=True, use_bg=True, legalize=True):
    """visits: list of (offset, width, jk) with jk in {1, 2} naming which T1
    chunk table the supertile's i1 one-hots select from."""
    dt = mybir.dt
    f32, bf16 = dt.float32, dt.bfloat16
    AF = mybir.ActivationFunctionType
    ALU = mybir.AluOpType

    nc = bass.Bass()

    fp8 = dt.float8e4
    # one-hot masks are exact in fp8: halves the startup-critical DMA bytes
    m0d = nc.dram_tensor("m0", [128, S], fp8, kind="ExternalInput")
    m1d = nc.dram_tensor("m1", [128, S], fp8, kind="ExternalInput")
    # embc[p, j, kc, v] = emb_tab(j)[chunk(j)*128 + v, kc*128 + p]
    embtd = nc.dram_tensor("embc", [128, 3, KC, 128], bf16, kind="ExternalInput")
    # w1m[t, fb, p, kc, ff] = W1flat[t*1024 + kc*128 + p, fb*1024 + ff]
    # (partition-major so one coarse dma_start covers a whole block)
    w1d = nc.dram_tensor("w1m", [2, FB, 128, KC, 1024], bf16, kind="ExternalInput")
    b1rd = nc.dram_tensor("b1row", [1, F], bf16, kind="ExternalInput")
    bgrd = nc.dram_tensor("bgrow", [1, E], bf16, kind="ExternalInput")
    wgd = nc.dram_tensor("wgm", [128, 2, KC, E], bf16, kind="ExternalInput")
    w2d = nc.dram_tensor("w2s", [128, E, KC, OUT], bf16, kind="ExternalInput")
    b2d = nc.dram_tensor("b2s", [128, E], f32, kind="ExternalInput")
    seld = nc.dram_tensor("sels", [128, E, 128], bf16, kind="ExternalInput")
    outd = nc.dram_tensor("out", [128, S], f32, kind="ExternalOutput")

    with tile.TileContext(nc) as tc:
        with (
            tc.tile_pool(name="const", bufs=1) as cpool,
            tc.tile_pool(name="w1st", bufs=2) as w1pool,
            tc.tile_pool(name="tt", bufs=2) as tpool,
            tc.tile_pool(name="hs", bufs=2) as hpool,
            tc.tile_pool(name="sm", bufs=2) as smpool,
            tc.tile_pool(name="gate", bufs=1) as gatepool,
            tc.tile_pool(name="accp", bufs=1) as apool,
            tc.tile_pool(name="tmpp", bufs=2) as tmpool,
            tc.tile_pool(name="gsc", bufs=2) as gspool,
            tc.tile_pool(name="pmm", bufs=2, space="PSUM") as pmm,
            tc.tile_pool(name="peo", bufs=1, space="PSUM") as peo,
            tc.tile_pool(name="pmisc", bufs=3, space="PSUM") as pmisc,
        ):
            # --- prologue DMAs, ordered by first use ---
            # COARSE dma_starts: each one costs ~600ns of Sync-engine issue
            # time, so the startup path uses few large transfers. The first
            # w1 piece + embc chunk 0 lead: they gate the first T matmuls.
            embc_sb = cpool.tile([128, 3, KC, 128], bf16)
            w1t0 = w1pool.tile([128, KC, 1024], bf16, tag="w1")
            nc.sync.dma_start(w1t0[:, 0:2, :], w1d[0, 0, :, 0:2])
            nc.sync.dma_start(embc_sb[:, 0], embtd[:, 0])
            wg_sb = cpool.tile([128, 2, KC, E], bf16)
            nc.sync.dma_start(wg_sb[:], wgd[:])
            bgr_sb = cpool.tile([1, E], bf16)
            nc.sync.dma_start(bgr_sb[:], bgrd[:])
            for k0 in range(2, KC, 2):
                nc.sync.dma_start(
                    w1t0[:, k0 : k0 + 2, :], w1d[0, 0, :, k0 : k0 + 2]
                )
            b1r_sb = cpool.tile([1, F], bf16)
            nc.sync.dma_start(b1r_sb[:], b1rd[:])

            ones128_bf = cpool.tile([1, 128], bf16)
            nc.vector.memset(ones128_bf[:], 1.0)
            # all-ones [128,128]: the sum-exp matmul then lands the sumexp
            # replicated on every output partition (broadcast for free)
            onessq = cpool.tile([128, 128], bf16)
            nc.vector.memset(onessq[:], 1.0)
            # exp'd gating chunk tables, padded to 128 stationary columns
            # (zero cols 4..127) so every main-loop matmul keeps the
            # (128,128) PE tile config — no quadrant-switch drains
            g128_sb = cpool.tile([128, 3, 128], bf16)
            nc.vector.memset(g128_sb[:], 0.0)

            # w1t1 + the remaining embc chunks + first mask pieces, ordered
            # by first use
            m0_sb = cpool.tile([128, S], fp8)
            m1_sb = cpool.tile([128, S], fp8)
            w1t1 = w1pool.tile([128, KC, 1024], bf16, tag="w1")
            nc.sync.dma_start(embc_sb[:, 1], embtd[:, 1])
            nc.sync.dma_start(w1t1[:, 0:4, :], w1d[1, 0, :, 0:4])
            nc.sync.dma_start(w1t1[:, 4:8, :], w1d[1, 0, :, 4:8])
            nc.sync.dma_start(embc_sb[:, 2], embtd[:, 2])
            mp = min(1024, S)
            nc.sync.dma_start(m0_sb[:, 0:mp], m0d[:, 0:mp])
            nc.sync.dma_start(m1_sb[:, 0:mp], m1d[:, 0:mp])

            def emit_g(j):
                # one Exp table chunk; all Exp run before any Silu so the ACT
                # table set loads exactly once each
                tj = 0 if j == 0 else 1
                psg = pmisc.tile([128, E], f32, tag="misc")
                for kc in range(KC):
                    nc.tensor.matmul(
                        psg[:],
                        embc_sb[:, j, kc, :],
                        wg_sb[:, tj, kc, :],
                        start=(kc == 0),
                        stop=(kc == KC - 1 and (j != 0 or not use_bg)),
                    )
                if j == 0 and use_bg:
                    # fold bg into chunk 0's table: psg += ones(v) x bg
                    nc.tensor.matmul(
                        psg[:], ones128_bf[:], bgr_sb[:], start=False, stop=True
                    )
                nc.scalar.activation(g128_sb[:, j, 0:E], psg[:], AF.Exp, bias=0.0)

            # W2 weights beat the mask remainder: the first visit's W2 runs at
            # ~31us while later mask pieces have ~10us of slack per piece
            w2_sb = cpool.tile([128, E, KC, OUT], bf16)
            nc.sync.dma_start(w2_sb[:], w2d[:])
            b2_sb = cpool.tile([128, E], f32)
            nc.sync.dma_start(b2_sb[:], b2d[:])
            sel_sb = cpool.tile([128, E, 128], bf16)
            nc.sync.dma_start(sel_sb[:], seld[:])

            for c0 in range(1024, S, 2048):
                c1 = min(S, c0 + 2048)
                nc.sync.dma_start(m0_sb[:, c0:c1], m0d[:, c0:c1])
                nc.sync.dma_start(m1_sb[:, c0:c1], m1d[:, c0:c1])

            acc = {}
            gates = {}
            gparts = {}

            def emit_gating_part1(vi, off, w, jk):
                # exp-gate selections land on psum rows 0..3 (zero-padded
                # stationary cols keep rows 4..127 at exactly 0)
                psa = pmisc.tile([128, w], f32, tag="misc")
                nc.tensor.matmul(
                    psa[:], g128_sb[:, 0, :], m0_sb[:, off : off + w],
                    start=True, stop=True,
                )
                psb = pmisc.tile([128, w], f32, tag="misc")
                nc.tensor.matmul(
                    psb[:], g128_sb[:, jk, :], m1_sb[:, off : off + w],
                    start=True, stop=True,
                )
                sa = smpool.tile([128, 512], f32, tag="s0")
                nc.vector.tensor_copy(sa[:, 0:w], psa[:])
                expt = smpool.tile([128, 512], bf16, tag="expt")
                nc.vector.tensor_tensor(expt[:, 0:w], sa[:, 0:w], psb[:], ALU.mult)
                gparts[vi] = expt

            def emit_gating_part2a(vi, off, w, jk):
                # sum-exp via all-ones stationary: sumexp appears replicated
                # on every psum row, so 1/sumexp needs no broadcast matmul
                expt = gparts[vi]
                sp = pmisc.tile([128, w], f32, tag="misc")
                nc.tensor.matmul(
                    sp[:], onessq[:], expt[:, 0:w], start=True, stop=True
                )
                rec = smpool.tile([128, 512], f32, tag="rec")
                nc.vector.reciprocal_approx_fast(rec[:, 0:w], sp[:])
                gparts[vi] = (expt, rec)

            def emit_gating_part2b(vi, off, w, jk):
                expt, rec = gparts.pop(vi)
                gt = gatepool.tile([128, 512], bf16, tag=f"gate{vi}")
                nc.vector.tensor_tensor(gt[:, 0:w], expt[:, 0:w], rec[:, 0:w], ALU.mult)
                gates[vi] = gt

            def emit_w2(eop, fb, w, hpair, p):
                for half in range(2):
                    dc = p * 2 + half
                    nc.tensor.matmul(
                        eop[:, 0:w],
                        w2_sb[:, fb, dc, :],
                        hpair[:, half, 0:w],
                        start=(dc == 0),
                        stop=(dc == KC - 1),
                    )

            def emit_combine(p):
                # (eop + b2) * gate -> acc, for the CARRIED visit p
                fb = p["fb"]
                vi, w, off, eop, gs = p["vi"], p["w"], p["off"], p["eop"], p["gs"]
                if fb == 0:
                    a = apool.tile([128, 512], f32, tag=f"acc{vi}")
                    acc[vi] = a
                    nc.vector.scalar_tensor_tensor(
                        a[:, 0:w], eop[:, 0:w], b2_sb[:, fb : fb + 1],
                        gs[:, 0:w], ALU.add, ALU.mult,
                    )
                else:
                    tmp = tmpool.tile([128, 512], f32, tag="tmp")
                    nc.vector.scalar_tensor_tensor(
                        tmp[:, 0:w], eop[:, 0:w], b2_sb[:, fb : fb + 1],
                        gs[:, 0:w], ALU.add, ALU.mult,
                    )
                    nc.vector.tensor_add(acc[vi][:, 0:w], acc[vi][:, 0:w], tmp[:, 0:w])
                if fb == FB - 1:
                    nc.sync.dma_start(outd[:, off : off + w], acc[vi][:, 0:w])

            def flush_pending(p):
                if p is None:
                    return
                emit_w2(p["eop"], p["fb"], p["w"], p["hs"][2], 2)
                emit_w2(p["eop"], p["fb"], p["w"], p["hs"][3], 3)
                emit_combine(p)

            def emit_visit(fb, vi, off, w, jk, t_sb, pending):
                # Software-pipelined across visits: this visit's selection
                # stream carries the PREVIOUS visit's last two W2 pairs and
                # combine, so every W2 matmul trails its silu evac by >2.5us
                # and the PE never waits on the ACT engine.
                if fb == 0:
                    emit_gating_part1(vi, off, w, jk)
                hs = []
                for pair in range(4):
                    hp = pmm.tile([128, 2, 512], f32, tag="mm")
                    # bank-interleaved order (T0h0, T0h1, T1h0, T1h1): every
                    # group-stop is followed by the other bank's stream, so
                    # the stop drain overlaps useful work
                    for half in range(2):
                        lf = pair * 2 + half
                        fs = (lf % 4) * 128
                        nc.tensor.matmul(
                            hp[:, half, 0:w],
                            t_sb[:, 0, lf // 4, fs : fs + 128],
                            m0_sb[:, off : off + w],
                            start=True, stop=False,
                        )
                    for half in range(2):
                        lf = pair * 2 + half
                        fs = (lf % 4) * 128
                        nc.tensor.matmul(
                            hp[:, half, 0:w],
                            t_sb[:, jk, lf // 4, fs : fs + 128],
                            m1_sb[:, off : off + w],
                            start=False, stop=True,
                        )
                    hpair = hpool.tile([128, 2, 512], bf16, tag=f"h{pair}")
                    nc.scalar.activation(
                        hpair[:, :, 0:w], hp[:, :, 0:w], AF.Silu, bias=0.0
                    )
                    hs.append(hpair)
                    if pair == 0 and pending is not None:
                        emit_w2(
                            pending["eop"], pending["fb"], pending["w"],
                            pending["hs"][2], 2,
                        )
                    if pair == 1 and pending is not None:
                        emit_w2(
                            pending["eop"], pending["fb"], pending["w"],
                            pending["hs"][3], 3,
                        )
                        emit_combine(pending)
                    if fb == 0 and pair == 1:
                        emit_gating_part2a(vi, off, w, jk)
                # in-visit W2 pairs 0/1 after the selection: maximum slack for
                # both the silu evacs and the peo buffer release (stt of the
                # carried visit fires ~2us before this point)
                eop = peo.tile([128, 512], f32, tag="eo")
                emit_w2(eop, fb, w, hs[0], 0)
                emit_w2(eop, fb, w, hs[1], 1)
                if fb == 0:
                    emit_gating_part2b(vi, off, w, jk)
                gp = pmisc.tile([128, w], f32, tag="misc")
                nc.tensor.matmul(
                    gp[:], sel_sb[:, fb, :], gates[vi][:, 0:w],
                    start=True, stop=True,
                )
                gs = gspool.tile([128, 512], bf16, tag="gs")
                nc.vector.tensor_copy(gs[:, 0:w], gp[:])
                return dict(vi=vi, off=off, w=w, eop=eop, hs=hs, gs=gs, fb=fb)

            # --- expert-phase-major main loop ---
            # `pending` (deferred W2 tail + combine) carries ACROSS phase
            # boundaries too: the next phase's precompute gives it slack
            pending = None
            for fb in range(FB):
                if fb == 0:
                    t0t, t1t = w1t0, w1t1
                else:
                    t0t = w1pool.tile([128, KC, 1024], bf16, tag="w1")
                    nc.sync.dma_start(t0t[:, 0:4, :], w1d[0, fb, :, 0:4])
                    nc.sync.dma_start(t0t[:, 4:8, :], w1d[0, fb, :, 4:8])
                    t1t = w1pool.tile([128, KC, 1024], bf16, tag="w1")
                    nc.sync.dma_start(t1t[:, 0:4, :], w1d[1, fb, :, 0:4])
                    nc.sync.dma_start(t1t[:, 4:8, :], w1d[1, fb, :, 4:8])
                # build the fb-slice of the 3 chunk tables
                t_sb = tpool.tile([128, 3, 2, 512], bf16, tag="t")

                def emit_tchunk(fb, j, w1t, t_sb):
                    ps = pmm.tile([128, 2, 512], f32, tag="mm")
                    for fp in range(2):
                        for kc in range(KC):
                            nc.tensor.matmul(
                                ps[:, fp, :],
                                embc_sb[:, j, kc, :],
                                w1t[:, kc, fp * 512 : (fp + 1) * 512],
                                start=(kc == 0),
                                stop=(kc == KC - 1 and (j != 0 or not use_b1)),
                            )
                        if j == 0 and use_b1:
                            f0 = fb * 1024 + fp * 512
                            nc.tensor.matmul(
                                ps[:, fp, :], ones128_bf[:],
                                b1r_sb[:, f0 : f0 + 512],
                                start=False, stop=True,
                            )
                    nc.scalar.copy(t_sb[:, j, :, :], ps[:])

                # bucket-b work (chunk-2 table, G2) is deferred past the
                # bucket-a visits: it is not needed until the first jk==2
                # visit, and in phase 0 this shortens the DMA-bound startup
                if fb == 0:
                    # G0 first: it needs only wg + embc chunk 0 (~0.3MB) and
                    # fills the PE while the first w1 block is still landing
                    emit_g(0)
                    emit_tchunk(fb, 0, t0t, t_sb)
                    emit_tchunk(fb, 1, t1t, t_sb)
                    emit_g(1)
                    emit_tchunk(fb, 2, t1t, t_sb)
                    emit_g(2)
                else:
                    for j in range(3):
                        emit_tchunk(fb, j, t0t if j == 0 else t1t, t_sb)
                for vi, (off, w, jk) in enumerate(visits):
                    pending = emit_visit(fb, vi, off, w, jk, t_sb, pending)
            flush_pending(pending)

    if legalize:
        _legalize_waits(nc)
    mybir.codegen_inst_isa_subclasses(nc)
    return nc


def _roundup(n, m):
    return -(-n // m) * m


def assign_slots(x):
    """Global (i0//128, i1//128) bucketing: 16 classes -> 8 cores (2 each,
    sharing the i0 chunk), with identical (i0, i1) token pairs DEDUPED —
    the device computes each distinct pair once and the host scatters the
    result to every duplicate token (~11% fewer slots on uniform data)."""
    x = np.asarray(x)
    c0 = x[:, 0] // 128
    c1 = x[:, 1] // 128
    key = x[:, 0] * V + x[:, 1]
    cores = []
    for g in range(NG):
        cls = []
        for b in range(NG):
            idx = np.nonzero((c0 == g) & (c1 == b))[0]
            ukey, inv = np.unique(key[idx], return_inverse=True)
            cls.append((idx, ukey, inv))
        order = sorted(range(NG), key=lambda b: -len(cls[b][1]))
        for pa, pb in ((order[0], order[3]), (order[1], order[2])):
            if len(cls[pb][1]) > len(cls[pa][1]):
                pa, pb = pb, pa
            cores.append(dict(c0=g, c1a=pa, c1b=pb, a=cls[pa], b=cls[pb]))
    SA = _roundup(max(len(c["a"][1]) for c in cores), 64)
    SB = _roundup(max(len(c["b"][1]) for c in cores), 64)
    visits = []
    off = 0
    for span, jk in ((SA, 1), (SB, 2)):
        left = span
        while left > 0:
            w = min(512, left)
            visits.append((off, w, jk))
            off += w
            left -= w
    S = SA + SB
    slot_pairs = []   # per core: slot -> (i0, i1), -1 for pad
    tok_maps = []     # per core: (global token ids, their slots)
    for c in cores:
        si0 = np.full(S, -1, dtype=np.int64)
        si1 = np.full(S, -1, dtype=np.int64)
        (ia, ka, va), (ib, kb, vb) = c["a"], c["b"]
        si0[0 : len(ka)] = ka // V
        si1[0 : len(ka)] = ka % V
        si0[SA : SA + len(kb)] = kb // V
        si1[SA : SA + len(kb)] = kb % V
        slot_pairs.append((si0, si1))
        tok_maps.append(
            (np.concatenate([ia, ib]), np.concatenate([va, SA + vb]))
        )
    return cores, slot_pairs, tok_maps, visits, S


def marshal_inputs(x, emb0, emb1, W1, b1, W2, b2, Wg, bg, cores, slot_pairs, S):
    x = np.asarray(x)
    emb0 = np.asarray(emb0)
    emb1 = np.asarray(emb1)

    shared = {}
    # W1flat[k, f] with f = e*1024 + d (expert-major features)
    w1flat = np.asarray(W1).transpose(1, 0, 2).reshape(IN, F)
    shared["w1m"] = np.ascontiguousarray(
        w1flat.reshape(2, KC, 128, FB, 1024).transpose(0, 3, 2, 1, 4).astype(BF16)
    )
    shared["b1row"] = np.ascontiguousarray(
        np.asarray(b1).reshape(1, F).astype(BF16)
    )
    shared["bgrow"] = np.ascontiguousarray(
        np.asarray(bg).reshape(1, E).astype(BF16)
    )
    shared["wgm"] = np.ascontiguousarray(
        np.asarray(Wg).reshape(2, KC, 128, E).transpose(2, 0, 1, 3).astype(BF16)
    )
    shared["w2s"] = np.ascontiguousarray(
        np.asarray(W2).reshape(E, KC, 128, OUT).transpose(2, 0, 1, 3).astype(BF16)
    )
    shared["b2s"] = np.ascontiguousarray(np.asarray(b2).T.astype(np.float32))
    # sel128[p, e, o] = 1 iff p == e: a (128,128)-tile gate-row broadcast
    sel128 = np.zeros((128, E, 128), dtype=BF16)
    for e in range(E):
        sel128[e, e, :] = 1.0
    shared["sels"] = np.ascontiguousarray(sel128)

    in_maps = []
    for c, (si0, si1) in zip(cores, slot_pairs):
        # embc[p, j, kc, v] = emb_tab(j)[chunk(j)*128 + v, kc*128 + p]
        embc = np.empty((128, 3, KC, 128), dtype=BF16)
        for j, (tab, ch) in enumerate(
            ((emb0, c["c0"]), (emb1, c["c1a"]), (emb1, c["c1b"]))
        ):
            chunk = tab[ch * 128 : (ch + 1) * 128]  # [128v, 1024k]
            embc[:, j] = (
                chunk.reshape(128, KC, 128).transpose(2, 1, 0).astype(BF16)
            )
        # one-hot masks per slot (pad slots stay all-zero); fp8 is exact
        m0 = np.zeros((128, S), dtype=FP8)
        m1 = np.zeros((128, S), dtype=FP8)
        pos = np.nonzero(si0 >= 0)[0]
        m0[si0[pos] % 128, pos] = 1.0
        m1[si1[pos] % 128, pos] = 1.0
        in_maps.append(
            {
                "m0": m0,
                "m1": m1,
                "embc": np.ascontiguousarray(embc),
                **shared,
            }
        )
    return in_maps


def kernel(x, emb0, emb1, W1, b1, W2, b2, Wg, bg):
    global LAST_EXEC_NS
    cores, slot_pairs, tok_maps, visits, S = assign_slots(x)
    # the fold matmuls for zero biases are pure overhead; the program is
    # built per call, so emit them only when actually needed
    nc = build_program(
        visits, S,
        use_b1=bool(np.any(np.asarray(b1))),
        use_bg=bool(np.any(np.asarray(bg))),
    )
    in_maps = marshal_inputs(
        x, emb0, emb1, W1, b1, W2, b2, Wg, bg, cores, slot_pairs, S
    )
    trace = os.environ.get("BASSMOE_TRACE", "0") == "1"
    res = run_bass_kernel_spmd(nc, in_maps, list(range(NCORES)), trace=trace)
    LAST_EXEC_NS = res.exec_time_ns
    out = np.empty((B, OUT), dtype=np.float32)
    for c in range(NCORES):
        toks, slots = tok_maps[c]
        r = res.results[c]["out"]  # [128, S]
        out[toks, :] = r[:, slots].T
    return out
